# revision 16
# baseline (speedup 1.0000x reference)
"""Trainium2 Bass kernel for a fused multi-head attention block.

Reference computation (B=4, T=2048, D=1152, H=8, HD=144, full rotary):
    q,k,v = x@Wq.T, x@Wk.T, x@Wv.T   (per head)
    q,k   = rope(q, k, cos, sin)
    o     = softmax(q k^T / sqrt(HD)) v
    out   = o @ Wo.T
Sharding (8 cores): core c = (batch b = c//2, head-group hg = c%2).
Each core computes 4 heads of one batch and a partial output
out_part = o_local @ Wo[:, hg_cols].T ; host sums the two partials per batch.

v2 design (vs. the transpose-heavy v1):
  * q/k are projected DIRECTLY transposed: qT = Wsel^T-chunks (stationary)
    x xT (moving), so the scores layout [dim, T] needs no PE transposes.
  * rope runs in the transposed layout: the rotate-half partner lives at a
    partition offset, so a DMA SBUF->SBUF copy builds a partition-shifted
    replica qS with qS[e] = q[partner(e)]; then
    q_rot = q * cosT + qS * sinT_signed  (3 bf16 tensor_tensor ops, the
    rotate-half signs are folded into sinT_signed on the host).
  * per-head packing (same as v1): heads' dims 0..127 in four [128,T] tiles,
    dims 128..143 in a shared b-block tile at rows 32h..32h+16 (+16 zero pad),
    replicated to all four 32-row groups so the K=32 score-tail matmuls can
    run concurrently via tile_position.
  * scores S^T accumulate in a [128,1024] PSUM tile (two banks, two
    key-tiles per group) so each Exp activation covers 1024 elems/partition —
    halving ScalarE instruction overhead vs. [128,512] activations.
  * softmax denominator via ones-column appended to v (o_ps[:,144]).
  * phase C (oT transpose + final projection) runs fully in bf16 and all
    PSUM->SBUF evacuations go to Vector/Scalar (never the same engine that
    gates the PE), keeping the PE stream dense so HAM stays at K=8/8.
  * 16 zero matmuls at t=0 prime the PE activity monitor while the first
    DMAs land; a dummy Exp preloads the ACT table set.
"""

import numpy as np

B, T, D, H = 4, 2048, 1152, 8
HL = 4              # heads per core
HD = 144            # head dim
EP = 640            # packed q/k width: 4*128 + 128 (4x(16+16pad))
DV = HL * HD        # 576, v/o width
VW = HL * (HD + 1)  # 580, v + ones col
NT = T // 128       # 16 t-tiles
KC = D // 128       # 9 contraction chunks
SCALE = float(HD) ** -0.5
NCORES = 8

_NC_CACHE = {}


def _build(debug=False):
    import concourse.bacc as bacc
    import concourse.mybir as mybir
    from concourse.tile import TileContext

    dt = mybir.dt
    f32, bf16 = dt.float32, dt.bfloat16
    AF = mybir.ActivationFunctionType

    nc = bacc.Bacc(
        "TRN2",
        target_bir_lowering=False,
        debug=debug,
        enable_asserts=False,
        num_devices=NCORES,
    )

    xT = nc.declare_dram_parameter("xT", [D, T], bf16, isOutput=False)
    wqT = nc.declare_dram_parameter("wqT", [D, EP], bf16, isOutput=False)
    wkT = nc.declare_dram_parameter("wkT", [D, EP], bf16, isOutput=False)
    wvT = nc.declare_dram_parameter("wvT", [D, DV], bf16, isOutput=False)
    woT = nc.declare_dram_parameter("woT", [DV, D], bf16, isOutput=False)
    cosTa = nc.declare_dram_parameter("cosTa", [128, T], bf16, isOutput=False)
    sinTa = nc.declare_dram_parameter("sinTa", [128, T], bf16, isOutput=False)
    cosTb = nc.declare_dram_parameter("cosTb", [128, T], bf16, isOutput=False)
    sinTb = nc.declare_dram_parameter("sinTb", [128, T], bf16, isOutput=False)
    identB = nc.declare_dram_parameter("identB", [128, 128], bf16, isOutput=False)
    out = nc.declare_dram_parameter("out", [T, D], f32, isOutput=True)

    with TileContext(nc) as tc:
        with tc.tile_pool(name="persist", bufs=1) as P0:
            ident_bf = P0.tile([128, 128], bf16, name="ident_bf", tag="ident_bf")
            nc.sync.dma_start(ident_bf[:], identB[:])

            qTa = [P0.tile([128, T], bf16, name=f"qTa{h}", tag=f"qTa{h}")
                   for h in range(HL)]
            kTa = [P0.tile([128, T], bf16, name=f"kTa{h}", tag=f"kTa{h}")
                   for h in range(HL)]
            qTBr = [P0.tile([128, T], bf16, name=f"qTBr{h}", tag=f"qTBr{h}")
                    for h in range(HL)]
            kTBr = [P0.tile([128, T], bf16, name=f"kTBr{h}", tag=f"kTBr{h}")
                    for h in range(HL)]
            vt = [P0.tile([128, VW], bf16, name=f"v{t}", tag=f"v{t}")
                  for t in range(NT)]

            # ------------- Phase A: transposed projections + rope ----------
            with (
                tc.tile_pool(name="pa", bufs=1) as pa,
                tc.tile_pool(name="paps", bufs=1, space="PSUM") as paps,
            ):
                # PE warm-up: zero matmuls keep the activity monitor busy
                # while the first weight/x DMAs land.
                wup = pa.tile([128, 512], bf16, name="wup", tag="wup")
                nc.vector.memset(wup[:], 0.0)
                for _ in range(16):
                    wps = paps.tile([128, 512], f32, name="wps", tag="wps", bufs=2)
                    nc.tensor.matmul(wps[:], wup[:, 0:128], wup[:],
                                     start=True, stop=True)
                # preload the exp table set early (one-time ~2.7us)
                dumm = pa.tile([128, 8], f32, name="dumm", tag="dumm")
                nc.scalar.activation(dumm[:], wup[:, 0:8], AF.Exp)

                xt = [pa.tile([128, T], bf16, name=f"xt{k}", tag=f"xt{k}")
                      for k in range(KC)]
                cos_a = pa.tile([128, T], bf16, name="cos_a", tag="cos_a")
                sin_a = pa.tile([128, T], bf16, name="sin_a", tag="sin_a")
                cos_b = pa.tile([128, T], bf16, name="cos_b", tag="cos_b")
                sin_b = pa.tile([128, T], bf16, name="sin_b", tag="sin_b")

                def qk_phase(wdram, dstA, dstBr, first=False):
                    wsb = []
                    for k in range(KC):
                        wt = pa.tile([128, EP], bf16, name=f"w{k}", tag=f"W{k}")
                        wsb.append(wt)
                    if first:
                        # (w_b[k], x[k] tg0) pairs land together so the first
                        # tg-outer B chain starts immediately and stays paced;
                        # w main-cols interleave after the first two tg sweeps
                        # (needed when the h-blocks start)
                        for k in range(KC):
                            nc.sync.dma_start(
                                wsb[k][:, 512:EP],
                                wdram[k * 128:(k + 1) * 128, 512:EP])
                            nc.sync.dma_start(
                                xt[k][:, 0:512], xT[k * 128:(k + 1) * 128, 0:512])
                        for k in range(KC):
                            nc.sync.dma_start(
                                xt[k][:, 512:1024],
                                xT[k * 128:(k + 1) * 128, 512:1024])
                        for k in range(KC):
                            nc.sync.dma_start(
                                wsb[k][:, 0:512],
                                wdram[k * 128:(k + 1) * 128, 0:512])
                        for j in range(2, 4):
                            for k in range(KC):
                                nc.sync.dma_start(
                                    xt[k][:, j * 512:(j + 1) * 512],
                                    xT[k * 128:(k + 1) * 128, j * 512:(j + 1) * 512])
                    else:
                        # per-128-col pieces, c-outer: write-after-read deps on
                        # the shared weight slots become monotone in time, so
                        # no piece head-of-line-blocks the sync DMA queue
                        for k in range(KC):
                            nc.sync.dma_start(
                                wsb[k][:, 512:EP],
                                wdram[k * 128:(k + 1) * 128, 512:EP])
                        for c in range(4):
                            for k in range(KC):
                                nc.sync.dma_start(
                                    wsb[k][:, c * 128:(c + 1) * 128],
                                    wdram[k * 128:(k + 1) * 128,
                                          c * 128:(c + 1) * 128])

                    def proj_block(cols, dst, tg_outer=False):
                        # dst (bf16 SBUF) <- (wsb[:, cols]).T @ xt
                        if tg_outer:
                            # consumes x tg-slice by tg-slice: right for the
                            # very first block while x is still streaming in
                            for tg in range(4):
                                ps = paps.tile([128, 512], f32, name="ppsp",
                                               tag=f"projps{tg}", bufs=1)
                                for k in range(KC):
                                    nc.tensor.matmul(
                                        ps[:], wsb[k][:, cols],
                                        xt[k][:, tg * 512:(tg + 1) * 512],
                                        start=(k == 0), stop=(k == KC - 1))
                                nc.scalar.copy(dst[:, tg * 512:(tg + 1) * 512],
                                               ps[:])
                            return
                        # k-outer: one stationary load feeds all 4 t-chains
                        pss = [paps.tile([128, 512], f32, name=f"pps{tg}",
                                         tag=f"projps{tg}", bufs=1)
                               for tg in range(4)]
                        for k in range(KC):
                            for tg in range(4):
                                nc.tensor.matmul(
                                    pss[tg][:], wsb[k][:, cols],
                                    xt[k][:, tg * 512:(tg + 1) * 512],
                                    start=(k == 0), stop=(k == KC - 1))
                        for tg in range(4):
                            nc.scalar.copy(dst[:, tg * 512:(tg + 1) * 512],
                                           pss[tg][:])

                    rawB = pa.tile([128, T], bf16, name="rawB", tag="rawB")
                    proj_block(slice(512, EP), rawB, tg_outer=first)
                    if first:
                        # trig loads delayed behind the B-block evac on the
                        # scalar FIFO so they don't steal HBM bandwidth from
                        # the critical first w/x loads
                        nc.scalar.dma_start(cos_a[:], cosTa[:])
                        nc.scalar.dma_start(sin_a[:], sinTa[:])
                        nc.scalar.dma_start(cos_b[:], cosTb[:])
                        nc.scalar.dma_start(sin_b[:], sinTb[:])
                    qSB = pa.tile([128, T], bf16, name="qSB", tag="qSB")
                    nc.gpsimd.memset(qSB[:], 0.0)

                    for h in range(HL):
                        rawA = pa.tile([128, T], bf16, name="rawA",
                                       tag="rawA", bufs=3)
                        proj_block(slice(h * 128, (h + 1) * 128), rawA)
                        # partition-shifted replica qS[e] = raw[partner(e)]
                        qS = pa.tile([128, T], bf16, name="qS", tag="qS", bufs=2)
                        nc.gpsimd.dma_start(qS[0:56, :], rawA[72:128, :])
                        nc.gpsimd.dma_start(qS[56:72, :], rawB[32 * h:32 * h + 16, :])
                        nc.gpsimd.dma_start(qS[72:128, :], rawA[0:56, :])
                        nc.gpsimd.dma_start(qSB[32 * h:32 * h + 16, :], rawA[56:72, :])
                        m1 = pa.tile([128, T], bf16, name="m1", tag="m1", bufs=2)
                        m2 = pa.tile([128, T], bf16, name="m2", tag="m2", bufs=2)
                        nc.vector.tensor_mul(m1[:], qS[:], sin_a[:])
                        nc.vector.tensor_mul(m2[:], rawA[:], cos_a[:])
                        nc.vector.tensor_add(dstA[h][:], m1[:], m2[:])

                    # b-block rope + 4x row-group replication
                    mB1 = pa.tile([128, T], bf16, name="mB1", tag="m1", bufs=2)
                    mB2 = pa.tile([128, T], bf16, name="mB2", tag="m2", bufs=2)
                    qTB = pa.tile([128, T], bf16, name="qTB", tag="qTB")
                    nc.vector.tensor_mul(mB1[:], qSB[:], sin_b[:])
                    nc.vector.tensor_mul(mB2[:], rawB[:], cos_b[:])
                    nc.vector.tensor_add(qTB[:], mB1[:], mB2[:])
                    for h in range(HL):
                        for j in range(4):
                            nc.gpsimd.dma_start(
                                dstBr[h][32 * j:32 * j + 32, :],
                                qTB[32 * h:32 * h + 32, :])

                # q first, then k, then v: each phase's b-block rope +
                # replication tail is hidden under the next phase's matmuls,
                # so the first attention group starts with all deps ready.
                qk_phase(wqT, qTa, qTBr, first=True)
                qk_phase(wkT, kTa, kTBr)

                # ---- v projection (natural [t, e] layout) ----
                wv_sb = []
                for k in range(KC):
                    wt = pa.tile([128, DV], bf16, name=f"wv{k}", tag=f"W{k}")
                    nc.sync.dma_start(wt[:], wvT[k * 128:(k + 1) * 128, :])
                    wv_sb.append(wt)
                for n in range(NT):
                    pg = 2 * (n % 2)   # alternate tag pairs = double buffering
                    ps0 = paps.tile([128, 288], f32, name="ps0",
                                    tag=f"projps{pg}", bufs=1)
                    ps1 = paps.tile([128, 288], f32, name="ps1",
                                    tag=f"projps{pg + 1}", bufs=1)
                    for k in range(KC):
                        lhs = xt[k][:, n * 128:(n + 1) * 128]
                        nc.tensor.matmul(ps0[:], lhs, wv_sb[k][:, 0:288],
                                         start=(k == 0), stop=(k == KC - 1))
                        nc.tensor.matmul(ps1[:], lhs, wv_sb[k][:, 288:DV],
                                         start=(k == 0), stop=(k == KC - 1))
                    v3 = vt[n].rearrange("p (h e) -> p h e", h=HL)
                    nc.scalar.copy(v3[:, 0:2, 0:HD],
                                   ps0.rearrange("p (h e) -> p h e", h=2))
                    nc.scalar.copy(v3[:, 2:4, 0:HD],
                                   ps1.rearrange("p (h e) -> p h e", h=2))
                    nc.vector.memset(v3[:, :, HD:HD + 1], 1.0)

            # ------------- Phase B: attention ------------------------------
            with tc.tile_pool(name="pb", bufs=1) as pb:
                ot = [pb.tile([128, DV], bf16, name=f"o{t}", tag=f"o{t}")
                      for t in range(NT)]
                with tc.tile_pool(name="pbps", bufs=1, space="PSUM") as pbps:
                    for qb in range(4):
                        for h in range(HL):
                            o_ps3 = pbps.tile([128, 3 * (HD + 1)], f32,
                                              name="o_ps3", tag="o3", bufs=2)
                            o_ps1 = pbps.tile([128, HD + 1], f32,
                                              name="o_ps1", tag="o1", bufs=2)
                            o_ps = [
                                o_ps3[:, 0:HD + 1],
                                o_ps3[:, HD + 1:2 * (HD + 1)],
                                o_ps3[:, 2 * (HD + 1):3 * (HD + 1)],
                                o_ps1[:],
                            ]

                            # key-tile groups of 2: one [128,1024] score-PSUM
                            # tile (2 banks) per group -> one Exp covers 1024
                            # elems/partition; with o3/o1 double-buffered the
                            # PSUM budget is 4+2+2 = 8 banks exactly
                            GRPS = [(2 * i, 2) for i in range(8)]

                            def s_exp(g):
                                kt0, gn = GRPS[g]
                                sps = pbps.tile([128, 1024], f32, name="sps",
                                                tag="sc", bufs=2)
                                # K=32 b-block tails first (start=True clears
                                # the bank), so the K=128 mains run
                                # back-to-back with stop=True
                                for j in range(gn):
                                    kt = kt0 + j
                                    rg = kt % 4
                                    nc.tensor.matmul(
                                        sps[:, j * 512:(j + 1) * 512],
                                        kTBr[h][32 * rg:32 * rg + 32,
                                                kt * 128:(kt + 1) * 128],
                                        qTBr[h][32 * rg:32 * rg + 32,
                                                qb * 512:(qb + 1) * 512],
                                        start=True, stop=False,
                                        tile_position=(32 * rg, 0))
                                for j in range(gn):
                                    kt = kt0 + j
                                    nc.tensor.matmul(
                                        sps[:, j * 512:(j + 1) * 512],
                                        kTa[h][:, kt * 128:(kt + 1) * 128],
                                        qTa[h][:, qb * 512:(qb + 1) * 512],
                                        start=False, stop=True)
                                E = pb.tile([128, 1024], bf16, name="E",
                                            tag="E", bufs=4)
                                nc.scalar.activation(E[:, 0:gn * 512],
                                                     sps[:, 0:gn * 512],
                                                     AF.Exp, scale=SCALE)
                                return E

                            def pv(g, E):
                                kt0, gn = GRPS[g]
                                for j in range(gn):
                                    kt = kt0 + j
                                    for qt in range(4):
                                        if qt < 3:
                                            st = kt == 0 and qt == 0
                                            sp = kt == NT - 1 and qt == 2
                                        else:
                                            st = kt == 0
                                            sp = kt == NT - 1
                                        nc.tensor.matmul(
                                            o_ps[qt][:],
                                            E[:, j * 512 + qt * 128:
                                              j * 512 + (qt + 1) * 128],
                                            vt[kt][:, (HD + 1) * h:
                                                   (HD + 1) * (h + 1)],
                                            start=st, stop=sp)

                            ngrp = len(GRPS)
                            Ep = s_exp(0)
                            for g in range(ngrp):
                                En = s_exp(g + 1) if g + 1 < ngrp else None
                                pv(g, Ep)
                                Ep = En
                            for qt in range(4):
                                t = qb * 4 + qt
                                r = pb.tile([128, 1], f32, name="r", tag="r",
                                            bufs=4)
                                nc.vector.reciprocal(r[:], o_ps[qt][:, HD:HD + 1])
                                nc.vector.tensor_scalar_mul(
                                    ot[t][:, HD * h:HD * (h + 1)],
                                    o_ps[qt][:, 0:HD], r[:])

                # ------------- Phase C: o^T + final projection -------------
                oTa = [pb.tile([128, T], bf16, name=f"oTa{j}", tag=f"oTa{j}")
                       for j in range(4)]
                oTb = pb.tile([64, T], bf16, name="oTb", tag="oTb")
                wo_sb = []
                for k in range(5):
                    rows = 128 if k < 4 else 64
                    wot = pb.tile([128, D], bf16, name=f"wo{k}", tag=f"wo{k}")
                    nc.sync.dma_start(wot[0:rows, :], woT[k * 128:k * 128 + rows, :])
                    wo_sb.append(wot)
                with tc.tile_pool(name="pcps", bufs=1, space="PSUM") as pcps:

                    def o_transp(t):
                        for j in range(4):
                            tp = pcps.tile([128, 128], bf16, name="tpo",
                                           tag="otp", bufs=4)
                            nc.tensor.transpose(
                                tp[:], ot[t][:, 128 * j:128 * (j + 1)],
                                ident_bf[:])
                            nc.scalar.copy(oTa[j][:, t * 128:(t + 1) * 128], tp[:])
                        tpb = pcps.tile([64, 128], bf16, name="tpb",
                                        tag="otp", bufs=4)
                        nc.tensor.transpose(tpb[:], ot[t][:, 512:DV], ident_bf[:])
                        nc.scalar.copy(oTb[:, t * 128:(t + 1) * 128], tpb[:])

                    def final(t):
                        for j3 in range(3):
                            fps = pcps.tile([128, 384], f32, name="fps",
                                            tag="f", bufs=3)
                            for k in range(5):
                                lhs = (oTa[k][:, t * 128:(t + 1) * 128]
                                       if k < 4
                                       else oTb[:, t * 128:(t + 1) * 128])
                                nc.tensor.matmul(
                                    fps[:], lhs,
                                    wo_sb[k][0:(128 if k < 4 else 64),
                                             384 * j3:384 * (j3 + 1)],
                                    start=(k == 0), stop=(k == 4))
                            fout = pb.tile([128, 384], f32, name="fout",
                                           tag="fout", bufs=4)
                            nc.vector.tensor_copy(fout[:], fps[:])
                            nc.sync.dma_start(
                                out[t * 128:(t + 1) * 128,
                                    384 * j3:384 * (j3 + 1)], fout[:])

                    o_transp(0)
                    for t in range(NT):
                        if t + 1 < NT:
                            o_transp(t + 1)
                        final(t)

    nc.compile()
    return nc


def get_nc(debug=False):
    key = bool(debug)
    if key not in _NC_CACHE:
        _NC_CACHE[key] = _build(debug)
    return _NC_CACHE[key]


def make_in_maps(x, cos, sin, Wq, Wk, Wv, Wo):
    import ml_dtypes

    x = np.asarray(x, np.float32)
    cos = np.asarray(cos, np.float32)
    sin = np.asarray(sin, np.float32)
    Wq, Wk, Wv, Wo = (np.asarray(w, np.float32) for w in (Wq, Wk, Wv, Wo))

    # transposed trig tables with the rotate-half signs folded in:
    # out[e] = raw[e]*cos[e] + sgn(e)*raw[partner(e)]*sin[e]
    cosT = np.ascontiguousarray(cos.T)   # [144, T]
    sinT = np.ascontiguousarray(sin.T)
    sgn = np.ones((128, 1), np.float32)
    sgn[:72] = -1.0
    cosTa = cosT[0:128]
    sinTa = sinT[0:128] * sgn
    cosTb = np.zeros((128, T), np.float32)
    sinTb = np.zeros((128, T), np.float32)
    for hh in range(HL):
        cosTb[32 * hh:32 * hh + 16] = cosT[128:144]
        sinTb[32 * hh:32 * hh + 16] = sinT[128:144]
    bf = ml_dtypes.bfloat16

    in_maps = []
    for c in range(NCORES):
        b, hg = divmod(c, 2)
        heads = [HL * hg + i for i in range(HL)]

        def qk_w(W):
            Wsel = np.zeros((EP, D), np.float32)
            for i, g in enumerate(heads):
                Wsel[128 * i:128 * i + 128] = W[144 * g:144 * g + 128]
                Wsel[512 + 32 * i:512 + 32 * i + 16] = W[144 * g + 128:144 * g + 144]
            return np.ascontiguousarray(Wsel.T)

        wv_sel = np.concatenate([Wv[144 * g:144 * g + 144] for g in heads], 0)
        wo_sel = np.concatenate([Wo[:, 144 * g:144 * g + 144] for g in heads], 1)
        in_maps.append(
            {
                "xT": np.ascontiguousarray(x[b].T).astype(bf),
                "wqT": qk_w(Wq).astype(bf),
                "wkT": qk_w(Wk).astype(bf),
                "wvT": np.ascontiguousarray(wv_sel.T).astype(bf),
                "woT": np.ascontiguousarray(wo_sel.T).astype(bf),
                "cosTa": cosTa.astype(bf),
                "sinTa": sinTa.astype(bf),
                "cosTb": cosTb.astype(bf),
                "sinTb": sinTb.astype(bf),
                "identB": np.eye(128, dtype=bf),
            }
        )
    return in_maps


def kernel(x, cos, sin, Wq, Wk, Wv, Wo, _trace=False, _trace_kwargs=None):
    from concourse.bass_utils import run_bass_kernel_spmd

    nc = get_nc()
    in_maps = make_in_maps(x, cos, sin, Wq, Wk, Wv, Wo)
    res = run_bass_kernel_spmd(
        nc,
        in_maps,
        list(range(NCORES)),
        trace=_trace,
        **(_trace_kwargs or {}),
    )
    parts = [res.results[c]["out"] for c in range(NCORES)]
    outb = np.stack([parts[2 * b] + parts[2 * b + 1] for b in range(B)])
    if _trace:
        kernel.last_results = res
    return outb.astype(np.float32)


# revision 18
# speedup vs baseline: 1.1287x; 1.1287x over previous
"""Trainium2 Bass kernel for a fused multi-head attention block.

Reference computation (B=4, T=2048, D=1152, H=8, HD=144, full rotary):
    q,k,v = x@Wq.T, x@Wk.T, x@Wv.T   (per head)
    q,k   = rope(q, k, cos, sin)
    o     = softmax(q k^T / sqrt(HD)) v
    out   = o @ Wo.T
Sharding (8 cores): core c = (batch b = c//2, head-group hg = c%2).
Each core computes 4 heads of one batch and a partial output
out_part = o_local @ Wo[:, hg_cols].T ; host sums the two partials per batch.

v2 design (vs. the transpose-heavy v1):
  * q/k are projected DIRECTLY transposed: qT = Wsel^T-chunks (stationary)
    x xT (moving), so the scores layout [dim, T] needs no PE transposes.
  * rope runs in the transposed layout: the rotate-half partner lives at a
    partition offset, so a DMA SBUF->SBUF copy builds a partition-shifted
    replica qS with qS[e] = q[partner(e)]; then
    q_rot = q * cosT + qS * sinT_signed  (3 bf16 tensor_tensor ops, the
    rotate-half signs are folded into sinT_signed on the host).
  * per-head packing (same as v1): heads' dims 0..127 in four [128,T] tiles,
    dims 128..143 in a shared b-block tile at rows 32h..32h+16 (+16 zero pad),
    replicated to all four 32-row groups so the K=32 score-tail matmuls can
    run concurrently via tile_position.
  * scores S^T accumulate in a [128,1024] PSUM tile (two banks, two
    key-tiles per group) so each Exp activation covers 1024 elems/partition —
    halving ScalarE instruction overhead vs. [128,512] activations.
  * softmax denominator via ones-column appended to v (o_ps[:,144]).
  * phase C (oT transpose + final projection) runs fully in bf16 and all
    PSUM->SBUF evacuations go to Vector/Scalar (never the same engine that
    gates the PE), keeping the PE stream dense so HAM stays at K=8/8.
  * 16 zero matmuls at t=0 prime the PE activity monitor while the first
    DMAs land; a dummy Exp preloads the ACT table set.
"""

import numpy as np

B, T, D, H = 4, 2048, 1152, 8
HL = 4              # heads per core
HD = 144            # head dim
EP = 640            # packed q/k width: 4*128 + 128 (4x(16+16pad))
DV = HL * HD        # 576, v/o width
VW = HL * (HD + 1)  # 580, v + ones col
NT = T // 128       # 16 t-tiles
KC = D // 128       # 9 contraction chunks
SCALE = float(HD) ** -0.5
NCORES = 8

_NC_CACHE = {}


def _build(debug=False):
    import concourse.bacc as bacc
    import concourse.mybir as mybir
    from concourse.tile import TileContext

    dt = mybir.dt
    f32, bf16 = dt.float32, dt.bfloat16
    AF = mybir.ActivationFunctionType

    nc = bacc.Bacc(
        "TRN2",
        target_bir_lowering=False,
        debug=debug,
        enable_asserts=False,
        num_devices=NCORES,
    )

    xT = nc.declare_dram_parameter("xT", [D, T], bf16, isOutput=False)
    wqT = nc.declare_dram_parameter("wqT", [D, EP], bf16, isOutput=False)
    wkT = nc.declare_dram_parameter("wkT", [D, EP], bf16, isOutput=False)
    wvT = nc.declare_dram_parameter("wvT", [D, DV], bf16, isOutput=False)
    woT = nc.declare_dram_parameter("woT", [DV, D], bf16, isOutput=False)
    cosTa = nc.declare_dram_parameter("cosTa", [128, T], bf16, isOutput=False)
    sinTa = nc.declare_dram_parameter("sinTa", [128, T], bf16, isOutput=False)
    cosTb = nc.declare_dram_parameter("cosTb", [128, T], bf16, isOutput=False)
    sinTb = nc.declare_dram_parameter("sinTb", [128, T], bf16, isOutput=False)
    identB = nc.declare_dram_parameter("identB", [128, 128], bf16, isOutput=False)
    out = nc.declare_dram_parameter("out", [T, D], f32, isOutput=True)

    with TileContext(nc) as tc:
        with tc.tile_pool(name="persist", bufs=1) as P0:
            ident_bf = P0.tile([128, 128], bf16, name="ident_bf", tag="ident_bf")
            nc.sync.dma_start(ident_bf[:], identB[:])

            qTa = [P0.tile([128, T], bf16, name=f"qTa{h}", tag=f"qTa{h}")
                   for h in range(HL)]
            kTa = [P0.tile([128, T], bf16, name=f"kTa{h}", tag=f"kTa{h}")
                   for h in range(HL)]
            qTBr = [P0.tile([128, T], bf16, name=f"qTBr{h}", tag=f"qTBr{h}")
                    for h in range(HL)]
            kTBr = [P0.tile([128, T], bf16, name=f"kTBr{h}", tag=f"kTBr{h}")
                    for h in range(HL)]
            vt = [P0.tile([128, VW], bf16, name=f"v{t}", tag=f"v{t}")
                  for t in range(NT)]

            # ------------- Phase A: transposed projections + rope ----------
            with (
                tc.tile_pool(name="pa", bufs=1) as pa,
                tc.tile_pool(name="paps", bufs=1, space="PSUM") as paps,
            ):
                # PE warm-up: zero matmuls keep the activity monitor busy
                # while the first weight/x DMAs land.
                wup = pa.tile([128, 512], bf16, name="wup", tag="wup")
                nc.vector.memset(wup[:], 0.0)
                for _ in range(16):
                    wps = paps.tile([128, 512], f32, name="wps", tag="wps", bufs=2)
                    nc.tensor.matmul(wps[:], wup[:, 0:128], wup[:],
                                     start=True, stop=True)
                # preload the exp table set early (one-time ~2.7us)
                dumm = pa.tile([128, 8], f32, name="dumm", tag="dumm")
                nc.scalar.activation(dumm[:], wup[:, 0:8], AF.Exp)

                xt = [pa.tile([128, T], bf16, name=f"xt{k}", tag=f"xt{k}")
                      for k in range(KC)]
                cos_a = pa.tile([128, T], bf16, name="cos_a", tag="cos_a")
                sin_a = pa.tile([128, T], bf16, name="sin_a", tag="sin_a")
                cos_b = pa.tile([128, T], bf16, name="cos_b", tag="cos_b")
                sin_b = pa.tile([128, T], bf16, name="sin_b", tag="sin_b")

                def qk_phase(wdram, dstA, dstBr, first=False):
                    wsb = []
                    for k in range(KC):
                        wt = pa.tile([128, EP], bf16, name=f"w{k}", tag=f"W{k}")
                        # b-cols first: the B block is projected first
                        nc.sync.dma_start(
                            wt[:, 512:EP], wdram[k * 128:(k + 1) * 128, 512:EP])
                        nc.sync.dma_start(
                            wt[:, 0:512], wdram[k * 128:(k + 1) * 128, 0:512])
                        wsb.append(wt)
                        if first:
                            # x chunk right after its weight chunk: the k-outer
                            # matmul order consumes (w[k], xt[k]) pairs in k
                            # order, so chains can start as soon as pair 0 lands
                            for j in range(4):
                                nc.sync.dma_start(
                                    xt[k][:, j * 512:(j + 1) * 512],
                                    xT[k * 128:(k + 1) * 128, j * 512:(j + 1) * 512])

                    def proj_block(cols, dst):
                        # dst (bf16 SBUF) <- (wsb[:, cols]).T @ xt
                        # k-outer: one stationary load feeds all 4 t-chains
                        pss = [paps.tile([128, 512], f32, name=f"pps{tg}",
                                         tag=f"projps{tg}", bufs=1)
                               for tg in range(4)]
                        for k in range(KC):
                            for tg in range(4):
                                nc.tensor.matmul(
                                    pss[tg][:], wsb[k][:, cols],
                                    xt[k][:, tg * 512:(tg + 1) * 512],
                                    start=(k == 0), stop=(k == KC - 1))
                        for tg in range(4):
                            nc.scalar.copy(dst[:, tg * 512:(tg + 1) * 512],
                                           pss[tg][:])

                    rawB = pa.tile([128, T], bf16, name="rawB", tag="rawB")
                    proj_block(slice(512, EP), rawB)
                    if first:
                        # trig loads delayed behind the B-block evac on the
                        # scalar FIFO so they don't steal HBM bandwidth from
                        # the critical first w/x loads
                        nc.scalar.dma_start(cos_a[:], cosTa[:])
                        nc.scalar.dma_start(sin_a[:], sinTa[:])
                        nc.scalar.dma_start(cos_b[:], cosTb[:])
                        nc.scalar.dma_start(sin_b[:], sinTb[:])
                    qSB = pa.tile([128, T], bf16, name="qSB", tag="qSB")
                    nc.gpsimd.memset(qSB[:], 0.0)

                    for h in range(HL):
                        rawA = pa.tile([128, T], bf16, name="rawA",
                                       tag="rawA", bufs=3)
                        proj_block(slice(h * 128, (h + 1) * 128), rawA)
                        # partition-shifted replica qS[e] = raw[partner(e)]
                        qS = pa.tile([128, T], bf16, name="qS", tag="qS", bufs=2)
                        nc.gpsimd.dma_start(qS[0:56, :], rawA[72:128, :])
                        nc.gpsimd.dma_start(qS[56:72, :], rawB[32 * h:32 * h + 16, :])
                        nc.gpsimd.dma_start(qS[72:128, :], rawA[0:56, :])
                        nc.gpsimd.dma_start(qSB[32 * h:32 * h + 16, :], rawA[56:72, :])
                        m1 = pa.tile([128, T], bf16, name="m1", tag="m1", bufs=2)
                        m2 = pa.tile([128, T], bf16, name="m2", tag="m2", bufs=2)
                        nc.vector.tensor_mul(m1[:], qS[:], sin_a[:])
                        nc.vector.tensor_mul(m2[:], rawA[:], cos_a[:])
                        nc.vector.tensor_add(dstA[h][:], m1[:], m2[:])

                    # b-block rope + 4x row-group replication
                    mB1 = pa.tile([128, T], bf16, name="mB1", tag="m1", bufs=2)
                    mB2 = pa.tile([128, T], bf16, name="mB2", tag="m2", bufs=2)
                    qTB = pa.tile([128, T], bf16, name="qTB", tag="qTB")
                    nc.vector.tensor_mul(mB1[:], qSB[:], sin_b[:])
                    nc.vector.tensor_mul(mB2[:], rawB[:], cos_b[:])
                    nc.vector.tensor_add(qTB[:], mB1[:], mB2[:])
                    for h in range(HL):
                        for j in range(4):
                            nc.gpsimd.dma_start(
                                dstBr[h][32 * j:32 * j + 32, :],
                                qTB[32 * h:32 * h + 32, :])

                # q first, then k, then v: each phase's b-block rope +
                # replication tail is hidden under the next phase's matmuls,
                # so the first attention group starts with all deps ready.
                qk_phase(wqT, qTa, qTBr, first=True)
                qk_phase(wkT, kTa, kTBr)

                # ---- v projection (natural [t, e] layout) ----
                wv_sb = []
                for k in range(KC):
                    wt = pa.tile([128, DV], bf16, name=f"wv{k}", tag=f"W{k}")
                    nc.sync.dma_start(wt[:], wvT[k * 128:(k + 1) * 128, :])
                    wv_sb.append(wt)
                for n in range(NT):
                    pg = 2 * (n % 2)   # alternate tag pairs = double buffering
                    ps0 = paps.tile([128, 288], f32, name="ps0",
                                    tag=f"projps{pg}", bufs=1)
                    ps1 = paps.tile([128, 288], f32, name="ps1",
                                    tag=f"projps{pg + 1}", bufs=1)
                    for k in range(KC):
                        lhs = xt[k][:, n * 128:(n + 1) * 128]
                        nc.tensor.matmul(ps0[:], lhs, wv_sb[k][:, 0:288],
                                         start=(k == 0), stop=(k == KC - 1))
                        nc.tensor.matmul(ps1[:], lhs, wv_sb[k][:, 288:DV],
                                         start=(k == 0), stop=(k == KC - 1))
                    v3 = vt[n].rearrange("p (h e) -> p h e", h=HL)
                    nc.scalar.copy(v3[:, 0:2, 0:HD],
                                   ps0.rearrange("p (h e) -> p h e", h=2))
                    nc.scalar.copy(v3[:, 2:4, 0:HD],
                                   ps1.rearrange("p (h e) -> p h e", h=2))
                    nc.vector.memset(v3[:, :, HD:HD + 1], 1.0)

            # ------------- Phase B: attention ------------------------------
            with tc.tile_pool(name="pb", bufs=1) as pb:
                ot = [pb.tile([128, DV], bf16, name=f"o{t}", tag=f"o{t}")
                      for t in range(NT)]
                with tc.tile_pool(name="pbps", bufs=1, space="PSUM") as pbps:
                    for qb in range(4):
                        for h in range(HL):
                            o_ps3 = pbps.tile([128, 3 * (HD + 1)], f32,
                                              name="o_ps3", tag="o3", bufs=2)
                            o_ps1 = pbps.tile([128, HD + 1], f32,
                                              name="o_ps1", tag="o1", bufs=2)
                            o_ps = [
                                o_ps3[:, 0:HD + 1],
                                o_ps3[:, HD + 1:2 * (HD + 1)],
                                o_ps3[:, 2 * (HD + 1):3 * (HD + 1)],
                                o_ps1[:],
                            ]

                            # key-tile groups of 2: one [128,1024] score-PSUM
                            # tile (2 banks) per group -> one Exp covers 1024
                            # elems/partition; with o3/o1 double-buffered the
                            # PSUM budget is 4+2+2 = 8 banks exactly
                            GRPS = [(2 * i, 2) for i in range(8)]

                            def s_exp(g):
                                kt0, gn = GRPS[g]
                                sps = pbps.tile([128, 1024], f32, name="sps",
                                                tag="sc", bufs=2)
                                # K=32 b-block tails first (start=True clears
                                # the bank), so the K=128 mains run
                                # back-to-back with stop=True
                                for j in range(gn):
                                    kt = kt0 + j
                                    rg = kt % 4
                                    nc.tensor.matmul(
                                        sps[:, j * 512:(j + 1) * 512],
                                        kTBr[h][32 * rg:32 * rg + 32,
                                                kt * 128:(kt + 1) * 128],
                                        qTBr[h][32 * rg:32 * rg + 32,
                                                qb * 512:(qb + 1) * 512],
                                        start=True, stop=False,
                                        tile_position=(32 * rg, 0))
                                for j in range(gn):
                                    kt = kt0 + j
                                    nc.tensor.matmul(
                                        sps[:, j * 512:(j + 1) * 512],
                                        kTa[h][:, kt * 128:(kt + 1) * 128],
                                        qTa[h][:, qb * 512:(qb + 1) * 512],
                                        start=False, stop=True)
                                E = pb.tile([128, 1024], bf16, name="E",
                                            tag="E", bufs=4)
                                nc.scalar.activation(E[:, 0:gn * 512],
                                                     sps[:, 0:gn * 512],
                                                     AF.Exp, scale=SCALE)
                                return E

                            def pv(g, E):
                                kt0, gn = GRPS[g]
                                for j in range(gn):
                                    kt = kt0 + j
                                    for qt in range(4):
                                        if qt < 3:
                                            st = kt == 0 and qt == 0
                                            sp = kt == NT - 1 and qt == 2
                                        else:
                                            st = kt == 0
                                            sp = kt == NT - 1
                                        nc.tensor.matmul(
                                            o_ps[qt][:],
                                            E[:, j * 512 + qt * 128:
                                              j * 512 + (qt + 1) * 128],
                                            vt[kt][:, (HD + 1) * h:
                                                   (HD + 1) * (h + 1)],
                                            start=st, stop=sp)

                            ngrp = len(GRPS)
                            Ep = s_exp(0)
                            for g in range(ngrp):
                                En = s_exp(g + 1) if g + 1 < ngrp else None
                                pv(g, Ep)
                                Ep = En
                            for qt in range(4):
                                t = qb * 4 + qt
                                r = pb.tile([128, 1], f32, name="r", tag="r",
                                            bufs=4)
                                nc.vector.reciprocal(r[:], o_ps[qt][:, HD:HD + 1])
                                nc.vector.tensor_scalar_mul(
                                    ot[t][:, HD * h:HD * (h + 1)],
                                    o_ps[qt][:, 0:HD], r[:])

                # ------------- Phase C: o^T + final projection -------------
                oTa = [pb.tile([128, T], bf16, name=f"oTa{j}", tag=f"oTa{j}")
                       for j in range(4)]
                oTb = pb.tile([64, T], bf16, name="oTb", tag="oTb")
                wo_sb = []
                for k in range(5):
                    rows = 128 if k < 4 else 64
                    wot = pb.tile([128, D], bf16, name=f"wo{k}", tag=f"wo{k}")
                    nc.sync.dma_start(wot[0:rows, :], woT[k * 128:k * 128 + rows, :])
                    wo_sb.append(wot)
                with tc.tile_pool(name="pcps", bufs=1, space="PSUM") as pcps:

                    def o_transp(t):
                        for j in range(4):
                            tp = pcps.tile([128, 128], bf16, name="tpo",
                                           tag="otp", bufs=4)
                            nc.tensor.transpose(
                                tp[:], ot[t][:, 128 * j:128 * (j + 1)],
                                ident_bf[:])
                            nc.scalar.copy(oTa[j][:, t * 128:(t + 1) * 128], tp[:])
                        tpb = pcps.tile([64, 128], bf16, name="tpb",
                                        tag="otp", bufs=4)
                        nc.tensor.transpose(tpb[:], ot[t][:, 512:DV], ident_bf[:])
                        nc.scalar.copy(oTb[:, t * 128:(t + 1) * 128], tpb[:])

                    def final(t):
                        for j3 in range(3):
                            fps = pcps.tile([128, 384], f32, name="fps",
                                            tag="f", bufs=3)
                            for k in range(5):
                                lhs = (oTa[k][:, t * 128:(t + 1) * 128]
                                       if k < 4
                                       else oTb[:, t * 128:(t + 1) * 128])
                                nc.tensor.matmul(
                                    fps[:], lhs,
                                    wo_sb[k][0:(128 if k < 4 else 64),
                                             384 * j3:384 * (j3 + 1)],
                                    start=(k == 0), stop=(k == 4))
                            fout = pb.tile([128, 384], f32, name="fout",
                                           tag="fout", bufs=4)
                            nc.vector.tensor_copy(fout[:], fps[:])
                            nc.sync.dma_start(
                                out[t * 128:(t + 1) * 128,
                                    384 * j3:384 * (j3 + 1)], fout[:])

                    o_transp(0)
                    for t in range(NT):
                        if t + 1 < NT:
                            o_transp(t + 1)
                        final(t)

    nc.compile()
    return nc


def get_nc(debug=False):
    key = bool(debug)
    if key not in _NC_CACHE:
        _NC_CACHE[key] = _build(debug)
    return _NC_CACHE[key]


def make_in_maps(x, cos, sin, Wq, Wk, Wv, Wo):
    import ml_dtypes

    x = np.asarray(x, np.float32)
    cos = np.asarray(cos, np.float32)
    sin = np.asarray(sin, np.float32)
    Wq, Wk, Wv, Wo = (np.asarray(w, np.float32) for w in (Wq, Wk, Wv, Wo))

    # transposed trig tables with the rotate-half signs folded in:
    # out[e] = raw[e]*cos[e] + sgn(e)*raw[partner(e)]*sin[e]
    cosT = np.ascontiguousarray(cos.T)   # [144, T]
    sinT = np.ascontiguousarray(sin.T)
    sgn = np.ones((128, 1), np.float32)
    sgn[:72] = -1.0
    cosTa = cosT[0:128]
    sinTa = sinT[0:128] * sgn
    cosTb = np.zeros((128, T), np.float32)
    sinTb = np.zeros((128, T), np.float32)
    for hh in range(HL):
        cosTb[32 * hh:32 * hh + 16] = cosT[128:144]
        sinTb[32 * hh:32 * hh + 16] = sinT[128:144]
    bf = ml_dtypes.bfloat16

    in_maps = []
    for c in range(NCORES):
        b, hg = divmod(c, 2)
        heads = [HL * hg + i for i in range(HL)]

        def qk_w(W):
            Wsel = np.zeros((EP, D), np.float32)
            for i, g in enumerate(heads):
                Wsel[128 * i:128 * i + 128] = W[144 * g:144 * g + 128]
                Wsel[512 + 32 * i:512 + 32 * i + 16] = W[144 * g + 128:144 * g + 144]
            return np.ascontiguousarray(Wsel.T)

        wv_sel = np.concatenate([Wv[144 * g:144 * g + 144] for g in heads], 0)
        wo_sel = np.concatenate([Wo[:, 144 * g:144 * g + 144] for g in heads], 1)
        in_maps.append(
            {
                "xT": np.ascontiguousarray(x[b].T).astype(bf),
                "wqT": qk_w(Wq).astype(bf),
                "wkT": qk_w(Wk).astype(bf),
                "wvT": np.ascontiguousarray(wv_sel.T).astype(bf),
                "woT": np.ascontiguousarray(wo_sel.T).astype(bf),
                "cosTa": cosTa.astype(bf),
                "sinTa": sinTa.astype(bf),
                "cosTb": cosTb.astype(bf),
                "sinTb": sinTb.astype(bf),
                "identB": np.eye(128, dtype=bf),
            }
        )
    return in_maps


def kernel(x, cos, sin, Wq, Wk, Wv, Wo, _trace=False, _trace_kwargs=None):
    from concourse.bass_utils import run_bass_kernel_spmd

    nc = get_nc()
    in_maps = make_in_maps(x, cos, sin, Wq, Wk, Wv, Wo)
    res = run_bass_kernel_spmd(
        nc,
        in_maps,
        list(range(NCORES)),
        trace=_trace,
        **(_trace_kwargs or {}),
    )
    parts = [res.results[c]["out"] for c in range(NCORES)]
    outb = np.stack([parts[2 * b] + parts[2 * b + 1] for b in range(B)])
    if _trace:
        kernel.last_results = res
    return outb.astype(np.float32)


# revision 22
# speedup vs baseline: 1.1328x; 1.0036x over previous
"""Trainium2 Bass kernel for a fused multi-head attention block.

Reference computation (B=4, T=2048, D=1152, H=8, HD=144, full rotary):
    q,k,v = x@Wq.T, x@Wk.T, x@Wv.T   (per head)
    q,k   = rope(q, k, cos, sin)
    o     = softmax(q k^T / sqrt(HD)) v
    out   = o @ Wo.T
Sharding (8 cores): core c = (batch b = c//2, head-group hg = c%2).
Each core computes 4 heads of one batch and a partial output
out_part = o_local @ Wo[:, hg_cols].T ; host sums the two partials per batch.

v2 design (vs. the transpose-heavy v1):
  * q/k are projected DIRECTLY transposed: qT = Wsel^T-chunks (stationary)
    x xT (moving), so the scores layout [dim, T] needs no PE transposes.
  * rope runs in the transposed layout: the rotate-half partner lives at a
    partition offset, so a DMA SBUF->SBUF copy builds a partition-shifted
    replica qS with qS[e] = q[partner(e)]; then
    q_rot = q * cosT + qS * sinT_signed  (3 bf16 tensor_tensor ops, the
    rotate-half signs are folded into sinT_signed on the host).
  * per-head packing (same as v1): heads' dims 0..127 in four [128,T] tiles,
    dims 128..143 in a shared b-block tile at rows 32h..32h+16 (+16 zero pad),
    replicated to all four 32-row groups so the K=32 score-tail matmuls can
    run concurrently via tile_position.
  * scores S^T accumulate in a [128,1024] PSUM tile (two banks, two
    key-tiles per group) so each Exp activation covers 1024 elems/partition —
    halving ScalarE instruction overhead vs. [128,512] activations.
  * softmax denominator via ones-column appended to v (o_ps[:,144]).
  * phase C (oT transpose + final projection) runs fully in bf16 and all
    PSUM->SBUF evacuations go to Vector/Scalar (never the same engine that
    gates the PE), keeping the PE stream dense so HAM stays at K=8/8.
  * 16 zero matmuls at t=0 prime the PE activity monitor while the first
    DMAs land; a dummy Exp preloads the ACT table set.
"""

import numpy as np

B, T, D, H = 4, 2048, 1152, 8
HL = 4              # heads per core
HD = 144            # head dim
EP = 640            # packed q/k width: 4*128 + 128 (4x(16+16pad))
DV = HL * HD        # 576, v/o width
VW = HL * (HD + 1)  # 580, v + ones col
NT = T // 128       # 16 t-tiles
KC = D // 128       # 9 contraction chunks
SCALE = float(HD) ** -0.5
NCORES = 8

_NC_CACHE = {}


def _build(debug=False):
    import concourse.bacc as bacc
    import concourse.mybir as mybir
    from concourse.tile import TileContext

    dt = mybir.dt
    f32, bf16 = dt.float32, dt.bfloat16
    AF = mybir.ActivationFunctionType

    nc = bacc.Bacc(
        "TRN2",
        target_bir_lowering=False,
        debug=debug,
        enable_asserts=False,
        num_devices=NCORES,
    )

    xT = nc.declare_dram_parameter("xT", [D, T], bf16, isOutput=False)
    wqT = nc.declare_dram_parameter("wqT", [D, EP], bf16, isOutput=False)
    wkT = nc.declare_dram_parameter("wkT", [D, EP], bf16, isOutput=False)
    wvT = nc.declare_dram_parameter("wvT", [D, DV], bf16, isOutput=False)
    woT = nc.declare_dram_parameter("woT", [DV, D], bf16, isOutput=False)
    cosTa = nc.declare_dram_parameter("cosTa", [128, T], bf16, isOutput=False)
    sinTa = nc.declare_dram_parameter("sinTa", [128, T], bf16, isOutput=False)
    cosTb = nc.declare_dram_parameter("cosTb", [128, T], bf16, isOutput=False)
    sinTb = nc.declare_dram_parameter("sinTb", [128, T], bf16, isOutput=False)
    identB = nc.declare_dram_parameter("identB", [128, 128], bf16, isOutput=False)
    out = nc.declare_dram_parameter("out", [T, D], f32, isOutput=True)

    with TileContext(nc) as tc:
        with tc.tile_pool(name="persist", bufs=1) as P0:
            ident_bf = P0.tile([128, 128], bf16, name="ident_bf", tag="ident_bf")
            nc.sync.dma_start(ident_bf[:], identB[:])

            qTa = [P0.tile([128, T], bf16, name=f"qTa{h}", tag=f"qTa{h}")
                   for h in range(HL)]
            kTa = [P0.tile([128, T], bf16, name=f"kTa{h}", tag=f"kTa{h}")
                   for h in range(HL)]
            qTBr = [P0.tile([128, T], bf16, name=f"qTBr{h}", tag=f"qTBr{h}")
                    for h in range(HL)]
            kTBr = [P0.tile([128, T], bf16, name=f"kTBr{h}", tag=f"kTBr{h}")
                    for h in range(HL)]
            vt = [P0.tile([128, VW], bf16, name=f"v{t}", tag=f"v{t}")
                  for t in range(NT)]

            # ------------- Phase A: transposed projections + rope ----------
            with (
                tc.tile_pool(name="pa", bufs=1) as pa,
                tc.tile_pool(name="paps", bufs=1, space="PSUM") as paps,
            ):
                # PE warm-up: zero matmuls keep the activity monitor busy
                # while the first weight/x DMAs land.
                wup = pa.tile([128, 512], bf16, name="wup", tag="wup")
                nc.vector.memset(wup[:], 0.0)
                for _ in range(16):
                    wps = paps.tile([128, 512], f32, name="wps", tag="wps", bufs=2)
                    nc.tensor.matmul(wps[:], wup[:, 0:128], wup[:],
                                     start=True, stop=True)
                # preload the exp table set early (one-time ~2.7us)
                dumm = pa.tile([128, 8], f32, name="dumm", tag="dumm")
                nc.scalar.activation(dumm[:], wup[:, 0:8], AF.Exp)

                xt = [pa.tile([128, T], bf16, name=f"xt{k}", tag=f"xt{k}")
                      for k in range(KC)]
                cos_a = pa.tile([128, T], bf16, name="cos_a", tag="cos_a")
                sin_a = pa.tile([128, T], bf16, name="sin_a", tag="sin_a")
                cos_b = pa.tile([128, T], bf16, name="cos_b", tag="cos_b")
                sin_b = pa.tile([128, T], bf16, name="sin_b", tag="sin_b")

                def qk_phase(wdram, dstA, dstBr, first=False):
                    wsb = []
                    for k in range(KC):
                        wt = pa.tile([128, EP], bf16, name=f"w{k}", tag=f"W{k}")
                        # b-cols first: the B block is projected first
                        nc.sync.dma_start(
                            wt[:, 512:EP], wdram[k * 128:(k + 1) * 128, 512:EP])
                        nc.sync.dma_start(
                            wt[:, 0:512], wdram[k * 128:(k + 1) * 128, 0:512])
                        wsb.append(wt)
                        if first:
                            # x chunk right after its weight chunk: the k-outer
                            # matmul order consumes (w[k], xt[k]) pairs in k
                            # order, so chains can start as soon as pair 0 lands
                            for j in range(4):
                                nc.sync.dma_start(
                                    xt[k][:, j * 512:(j + 1) * 512],
                                    xT[k * 128:(k + 1) * 128, j * 512:(j + 1) * 512])

                    def proj_block(cols, dst, sprinkle=False):
                        # dst (bf16 SBUF) <- (wsb[:, cols]).T @ xt
                        # k-outer: one stationary load feeds all 4 t-chains
                        pss = [paps.tile([128, 512], f32, name=f"pps{tg}",
                                         tag=f"projps{tg}", bufs=1)
                               for tg in range(4)]
                        for k in range(KC):
                            for tg in range(4):
                                nc.tensor.matmul(
                                    pss[tg][:], wsb[k][:, cols],
                                    xt[k][:, tg * 512:(tg + 1) * 512],
                                    start=(k == 0), stop=(k == KC - 1))
                            if sprinkle and k % 2 == 0:
                                # dep-free dummy MM: during the DMA-gated ramp
                                # these fill PE idle windows so the HAM clock
                                # gate never re-throttles to K=4/8
                                wps = paps.tile([128, 512], f32, name="wps",
                                                tag="wps", bufs=2)
                                nc.tensor.matmul(wps[:], wup[:, 0:128], wup[:],
                                                 start=True, stop=True)
                        for tg in range(4):
                            nc.scalar.copy(dst[:, tg * 512:(tg + 1) * 512],
                                           pss[tg][:])

                    rawB = pa.tile([128, T], bf16, name="rawB", tag="rawB")
                    proj_block(slice(512, EP), rawB, sprinkle=first)
                    if first:
                        # trig loads delayed behind the B-block evac on the
                        # scalar FIFO so they don't steal HBM bandwidth from
                        # the critical first w/x loads
                        nc.scalar.dma_start(cos_a[:], cosTa[:])
                        nc.scalar.dma_start(sin_a[:], sinTa[:])
                        nc.scalar.dma_start(cos_b[:], cosTb[:])
                        nc.scalar.dma_start(sin_b[:], sinTb[:])
                    qSB = pa.tile([128, T], bf16, name="qSB", tag="qSB")
                    nc.gpsimd.memset(qSB[:], 0.0)

                    for h in range(HL):
                        rawA = pa.tile([128, T], bf16, name="rawA",
                                       tag="rawA", bufs=3)
                        proj_block(slice(h * 128, (h + 1) * 128), rawA,
                                   sprinkle=(first and h < 2))
                        # partition-shifted replica qS[e] = raw[partner(e)]
                        qS = pa.tile([128, T], bf16, name="qS", tag="qS", bufs=2)
                        nc.gpsimd.dma_start(qS[0:56, :], rawA[72:128, :])
                        nc.gpsimd.dma_start(qS[56:72, :], rawB[32 * h:32 * h + 16, :])
                        nc.gpsimd.dma_start(qS[72:128, :], rawA[0:56, :])
                        nc.gpsimd.dma_start(qSB[32 * h:32 * h + 16, :], rawA[56:72, :])
                        m1 = pa.tile([128, T], bf16, name="m1", tag="m1", bufs=2)
                        m2 = pa.tile([128, T], bf16, name="m2", tag="m2", bufs=2)
                        nc.vector.tensor_mul(m1[:], qS[:], sin_a[:])
                        nc.vector.tensor_mul(m2[:], rawA[:], cos_a[:])
                        nc.vector.tensor_add(dstA[h][:], m1[:], m2[:])

                    # b-block rope + 4x row-group replication
                    mB1 = pa.tile([128, T], bf16, name="mB1", tag="m1", bufs=2)
                    mB2 = pa.tile([128, T], bf16, name="mB2", tag="m2", bufs=2)
                    qTB = pa.tile([128, T], bf16, name="qTB", tag="qTB")
                    nc.vector.tensor_mul(mB1[:], qSB[:], sin_b[:])
                    nc.vector.tensor_mul(mB2[:], rawB[:], cos_b[:])
                    nc.vector.tensor_add(qTB[:], mB1[:], mB2[:])
                    for h in range(HL):
                        for j in range(4):
                            nc.gpsimd.dma_start(
                                dstBr[h][32 * j:32 * j + 32, :],
                                qTB[32 * h:32 * h + 32, :])

                # q first, then k, then v: each phase's b-block rope +
                # replication tail is hidden under the next phase's matmuls,
                # so the first attention group starts with all deps ready.
                qk_phase(wqT, qTa, qTBr, first=True)
                qk_phase(wkT, kTa, kTBr)

                # ---- v projection (natural [t, e] layout) ----
                wv_sb = []
                for k in range(KC):
                    wt = pa.tile([128, DV], bf16, name=f"wv{k}", tag=f"W{k}")
                    nc.sync.dma_start(wt[:], wvT[k * 128:(k + 1) * 128, :])
                    wv_sb.append(wt)
                for n in range(NT):
                    pg = 2 * (n % 2)   # alternate tag pairs = double buffering
                    ps0 = paps.tile([128, 288], f32, name="ps0",
                                    tag=f"projps{pg}", bufs=1)
                    ps1 = paps.tile([128, 288], f32, name="ps1",
                                    tag=f"projps{pg + 1}", bufs=1)
                    for k in range(KC):
                        lhs = xt[k][:, n * 128:(n + 1) * 128]
                        nc.tensor.matmul(ps0[:], lhs, wv_sb[k][:, 0:288],
                                         start=(k == 0), stop=(k == KC - 1))
                        nc.tensor.matmul(ps1[:], lhs, wv_sb[k][:, 288:DV],
                                         start=(k == 0), stop=(k == KC - 1))
                    v3 = vt[n].rearrange("p (h e) -> p h e", h=HL)
                    nc.scalar.copy(v3[:, 0:2, 0:HD],
                                   ps0.rearrange("p (h e) -> p h e", h=2))
                    nc.scalar.copy(v3[:, 2:4, 0:HD],
                                   ps1.rearrange("p (h e) -> p h e", h=2))
                    nc.vector.memset(v3[:, :, HD:HD + 1], 1.0)

            # ------------- Phase B: attention ------------------------------
            with tc.tile_pool(name="pb", bufs=1) as pb:
                ot = [pb.tile([128, DV], bf16, name=f"o{t}", tag=f"o{t}")
                      for t in range(NT)]
                with tc.tile_pool(name="pbps", bufs=1, space="PSUM") as pbps:
                    for qb in range(4):
                        for h in range(HL):
                            o_ps3 = pbps.tile([128, 3 * (HD + 1)], f32,
                                              name="o_ps3", tag="o3", bufs=2)
                            o_ps1 = pbps.tile([128, HD + 1], f32,
                                              name="o_ps1", tag="o1", bufs=2)
                            o_ps = [
                                o_ps3[:, 0:HD + 1],
                                o_ps3[:, HD + 1:2 * (HD + 1)],
                                o_ps3[:, 2 * (HD + 1):3 * (HD + 1)],
                                o_ps1[:],
                            ]

                            # key-tile groups of 2: one [128,1024] score-PSUM
                            # tile (2 banks) per group -> one Exp covers 1024
                            # elems/partition; with o3/o1 double-buffered the
                            # PSUM budget is 4+2+2 = 8 banks exactly
                            GRPS = [(2 * i, 2) for i in range(8)]

                            def s_exp(g):
                                kt0, gn = GRPS[g]
                                sps = pbps.tile([128, 1024], f32, name="sps",
                                                tag="sc", bufs=2)
                                # K=32 b-block tails first (start=True clears
                                # the bank), so the K=128 mains run
                                # back-to-back with stop=True
                                for j in range(gn):
                                    kt = kt0 + j
                                    rg = kt % 4
                                    nc.tensor.matmul(
                                        sps[:, j * 512:(j + 1) * 512],
                                        kTBr[h][32 * rg:32 * rg + 32,
                                                kt * 128:(kt + 1) * 128],
                                        qTBr[h][32 * rg:32 * rg + 32,
                                                qb * 512:(qb + 1) * 512],
                                        start=True, stop=False,
                                        tile_position=(32 * rg, 0))
                                for j in range(gn):
                                    kt = kt0 + j
                                    nc.tensor.matmul(
                                        sps[:, j * 512:(j + 1) * 512],
                                        kTa[h][:, kt * 128:(kt + 1) * 128],
                                        qTa[h][:, qb * 512:(qb + 1) * 512],
                                        start=False, stop=True)
                                E = pb.tile([128, 1024], bf16, name="E",
                                            tag="E", bufs=4)
                                nc.scalar.activation(E[:, 0:gn * 512],
                                                     sps[:, 0:gn * 512],
                                                     AF.Exp, scale=SCALE)
                                return E

                            def pv(g, E):
                                kt0, gn = GRPS[g]
                                for j in range(gn):
                                    kt = kt0 + j
                                    for qt in range(4):
                                        if qt < 3:
                                            st = kt == 0 and qt == 0
                                            sp = kt == NT - 1 and qt == 2
                                        else:
                                            st = kt == 0
                                            sp = kt == NT - 1
                                        nc.tensor.matmul(
                                            o_ps[qt][:],
                                            E[:, j * 512 + qt * 128:
                                              j * 512 + (qt + 1) * 128],
                                            vt[kt][:, (HD + 1) * h:
                                                   (HD + 1) * (h + 1)],
                                            start=st, stop=sp)

                            ngrp = len(GRPS)
                            Ep = s_exp(0)
                            for g in range(ngrp):
                                En = s_exp(g + 1) if g + 1 < ngrp else None
                                pv(g, Ep)
                                Ep = En
                            for qt in range(4):
                                t = qb * 4 + qt
                                r = pb.tile([128, 1], f32, name="r", tag="r",
                                            bufs=4)
                                nc.vector.reciprocal(r[:], o_ps[qt][:, HD:HD + 1])
                                nc.vector.tensor_scalar_mul(
                                    ot[t][:, HD * h:HD * (h + 1)],
                                    o_ps[qt][:, 0:HD], r[:])

                # ------------- Phase C: o^T + final projection -------------
                oTa = [pb.tile([128, T], bf16, name=f"oTa{j}", tag=f"oTa{j}")
                       for j in range(4)]
                oTb = pb.tile([64, T], bf16, name="oTb", tag="oTb")
                wo_sb = []
                for k in range(5):
                    rows = 128 if k < 4 else 64
                    wot = pb.tile([128, D], bf16, name=f"wo{k}", tag=f"wo{k}")
                    nc.sync.dma_start(wot[0:rows, :], woT[k * 128:k * 128 + rows, :])
                    wo_sb.append(wot)
                with tc.tile_pool(name="pcps", bufs=1, space="PSUM") as pcps:

                    def o_transp(t):
                        for j in range(4):
                            tp = pcps.tile([128, 128], bf16, name="tpo",
                                           tag="otp", bufs=4)
                            nc.tensor.transpose(
                                tp[:], ot[t][:, 128 * j:128 * (j + 1)],
                                ident_bf[:])
                            nc.scalar.copy(oTa[j][:, t * 128:(t + 1) * 128], tp[:])
                        tpb = pcps.tile([64, 128], bf16, name="tpb",
                                        tag="otp", bufs=4)
                        nc.tensor.transpose(tpb[:], ot[t][:, 512:DV], ident_bf[:])
                        nc.scalar.copy(oTb[:, t * 128:(t + 1) * 128], tpb[:])

                    def final(t):
                        for j3 in range(3):
                            fps = pcps.tile([128, 384], f32, name="fps",
                                            tag="f", bufs=4)
                            for k in range(5):
                                lhs = (oTa[k][:, t * 128:(t + 1) * 128]
                                       if k < 4
                                       else oTb[:, t * 128:(t + 1) * 128])
                                nc.tensor.matmul(
                                    fps[:], lhs,
                                    wo_sb[k][0:(128 if k < 4 else 64),
                                             384 * j3:384 * (j3 + 1)],
                                    start=(k == 0), stop=(k == 4))
                            fout = pb.tile([128, 384], f32, name="fout",
                                           tag="fout", bufs=6)
                            nc.vector.tensor_copy(fout[:], fps[:])
                            nc.sync.dma_start(
                                out[t * 128:(t + 1) * 128,
                                    384 * j3:384 * (j3 + 1)], fout[:])

                    o_transp(0)
                    for t in range(NT):
                        if t + 1 < NT:
                            o_transp(t + 1)
                        final(t)

    nc.compile()
    return nc


def get_nc(debug=False):
    key = bool(debug)
    if key not in _NC_CACHE:
        _NC_CACHE[key] = _build(debug)
    return _NC_CACHE[key]


def make_in_maps(x, cos, sin, Wq, Wk, Wv, Wo):
    import ml_dtypes

    x = np.asarray(x, np.float32)
    cos = np.asarray(cos, np.float32)
    sin = np.asarray(sin, np.float32)
    Wq, Wk, Wv, Wo = (np.asarray(w, np.float32) for w in (Wq, Wk, Wv, Wo))

    # transposed trig tables with the rotate-half signs folded in:
    # out[e] = raw[e]*cos[e] + sgn(e)*raw[partner(e)]*sin[e]
    cosT = np.ascontiguousarray(cos.T)   # [144, T]
    sinT = np.ascontiguousarray(sin.T)
    sgn = np.ones((128, 1), np.float32)
    sgn[:72] = -1.0
    cosTa = cosT[0:128]
    sinTa = sinT[0:128] * sgn
    cosTb = np.zeros((128, T), np.float32)
    sinTb = np.zeros((128, T), np.float32)
    for hh in range(HL):
        cosTb[32 * hh:32 * hh + 16] = cosT[128:144]
        sinTb[32 * hh:32 * hh + 16] = sinT[128:144]
    bf = ml_dtypes.bfloat16

    in_maps = []
    for c in range(NCORES):
        b, hg = divmod(c, 2)
        heads = [HL * hg + i for i in range(HL)]

        def qk_w(W):
            Wsel = np.zeros((EP, D), np.float32)
            for i, g in enumerate(heads):
                Wsel[128 * i:128 * i + 128] = W[144 * g:144 * g + 128]
                Wsel[512 + 32 * i:512 + 32 * i + 16] = W[144 * g + 128:144 * g + 144]
            return np.ascontiguousarray(Wsel.T)

        wv_sel = np.concatenate([Wv[144 * g:144 * g + 144] for g in heads], 0)
        wo_sel = np.concatenate([Wo[:, 144 * g:144 * g + 144] for g in heads], 1)
        in_maps.append(
            {
                "xT": np.ascontiguousarray(x[b].T).astype(bf),
                "wqT": qk_w(Wq).astype(bf),
                "wkT": qk_w(Wk).astype(bf),
                "wvT": np.ascontiguousarray(wv_sel.T).astype(bf),
                "woT": np.ascontiguousarray(wo_sel.T).astype(bf),
                "cosTa": cosTa.astype(bf),
                "sinTa": sinTa.astype(bf),
                "cosTb": cosTb.astype(bf),
                "sinTb": sinTb.astype(bf),
                "identB": np.eye(128, dtype=bf),
            }
        )
    return in_maps


def kernel(x, cos, sin, Wq, Wk, Wv, Wo, _trace=False, _trace_kwargs=None):
    from concourse.bass_utils import run_bass_kernel_spmd

    nc = get_nc()
    in_maps = make_in_maps(x, cos, sin, Wq, Wk, Wv, Wo)
    res = run_bass_kernel_spmd(
        nc,
        in_maps,
        list(range(NCORES)),
        trace=_trace,
        **(_trace_kwargs or {}),
    )
    parts = [res.results[c]["out"] for c in range(NCORES)]
    outb = np.stack([parts[2 * b] + parts[2 * b + 1] for b in range(B)])
    if _trace:
        kernel.last_results = res
    return outb.astype(np.float32)


# revision 23
# speedup vs baseline: 1.1600x; 1.0241x over previous
"""Trainium2 Bass kernel for a fused multi-head attention block.

Reference computation (B=4, T=2048, D=1152, H=8, HD=144, full rotary):
    q,k,v = x@Wq.T, x@Wk.T, x@Wv.T   (per head)
    q,k   = rope(q, k, cos, sin)
    o     = softmax(q k^T / sqrt(HD)) v
    out   = o @ Wo.T
Sharding (8 cores): core c = (batch b = c//2, head-group hg = c%2).
Each core computes 4 heads of one batch and a partial output
out_part = o_local @ Wo[:, hg_cols].T ; host sums the two partials per batch.

v2 design (vs. the transpose-heavy v1):
  * q/k are projected DIRECTLY transposed: qT = Wsel^T-chunks (stationary)
    x xT (moving), so the scores layout [dim, T] needs no PE transposes.
  * rope runs in the transposed layout: the rotate-half partner lives at a
    partition offset, so a DMA SBUF->SBUF copy builds a partition-shifted
    replica qS with qS[e] = q[partner(e)]; then
    q_rot = q * cosT + qS * sinT_signed  (3 bf16 tensor_tensor ops, the
    rotate-half signs are folded into sinT_signed on the host).
  * per-head packing (same as v1): heads' dims 0..127 in four [128,T] tiles,
    dims 128..143 in a shared b-block tile at rows 32h..32h+16 (+16 zero pad),
    replicated to all four 32-row groups so the K=32 score-tail matmuls can
    run concurrently via tile_position.
  * scores S^T accumulate in a [128,1024] PSUM tile (two banks, two
    key-tiles per group) so each Exp activation covers 1024 elems/partition —
    halving ScalarE instruction overhead vs. [128,512] activations.
  * softmax denominator via ones-column appended to v (o_ps[:,144]).
  * phase C (oT transpose + final projection) runs fully in bf16 and all
    PSUM->SBUF evacuations go to Vector/Scalar (never the same engine that
    gates the PE), keeping the PE stream dense so HAM stays at K=8/8.
  * 16 zero matmuls at t=0 prime the PE activity monitor while the first
    DMAs land; a dummy Exp preloads the ACT table set.
"""

import numpy as np

B, T, D, H = 4, 2048, 1152, 8
HL = 4              # heads per core
HD = 144            # head dim
EP = 640            # packed q/k width: 4*128 + 128 (4x(16+16pad))
DV = HL * HD        # 576, v/o width
VW = HL * (HD + 1)  # 580, v + ones col
NT = T // 128       # 16 t-tiles
KC = D // 128       # 9 contraction chunks
SCALE = float(HD) ** -0.5
NCORES = 8

_NC_CACHE = {}


def _build(debug=False):
    import concourse.bacc as bacc
    import concourse.mybir as mybir
    from concourse.tile import TileContext

    dt = mybir.dt
    f32, bf16 = dt.float32, dt.bfloat16
    AF = mybir.ActivationFunctionType

    nc = bacc.Bacc(
        "TRN2",
        target_bir_lowering=False,
        debug=debug,
        enable_asserts=False,
        num_devices=NCORES,
    )

    xT = nc.declare_dram_parameter("xT", [D, T], bf16, isOutput=False)
    wqT = nc.declare_dram_parameter("wqT", [D, EP], bf16, isOutput=False)
    wkT = nc.declare_dram_parameter("wkT", [D, EP], bf16, isOutput=False)
    wvT = nc.declare_dram_parameter("wvT", [D, DV], bf16, isOutput=False)
    woT = nc.declare_dram_parameter("woT", [DV, D], bf16, isOutput=False)
    cosTa = nc.declare_dram_parameter("cosTa", [128, T], bf16, isOutput=False)
    sinTa = nc.declare_dram_parameter("sinTa", [128, T], bf16, isOutput=False)
    cosTb = nc.declare_dram_parameter("cosTb", [128, T], bf16, isOutput=False)
    sinTb = nc.declare_dram_parameter("sinTb", [128, T], bf16, isOutput=False)
    identB = nc.declare_dram_parameter("identB", [128, 128], bf16, isOutput=False)
    out = nc.declare_dram_parameter("out", [T, D], f32, isOutput=True)

    with TileContext(nc) as tc:
        with tc.tile_pool(name="persist", bufs=1) as P0:
            ident_bf = P0.tile([128, 128], bf16, name="ident_bf", tag="ident_bf")
            nc.sync.dma_start(ident_bf[:], identB[:])

            qTa = [P0.tile([128, T], bf16, name=f"qTa{h}", tag=f"qTa{h}")
                   for h in range(HL)]
            kTa = [P0.tile([128, T], bf16, name=f"kTa{h}", tag=f"kTa{h}")
                   for h in range(HL)]
            qTBr = [P0.tile([128, T], bf16, name=f"qTBr{h}", tag=f"qTBr{h}")
                    for h in range(HL)]
            kTBr = [P0.tile([128, T], bf16, name=f"kTBr{h}", tag=f"kTBr{h}")
                    for h in range(HL)]
            vt = [P0.tile([128, VW], bf16, name=f"v{t}", tag=f"v{t}")
                  for t in range(NT)]

            # ------------- Phase A: transposed projections + rope ----------
            with (
                tc.tile_pool(name="pa", bufs=1) as pa,
                tc.tile_pool(name="paps", bufs=1, space="PSUM") as paps,
            ):
                # PE warm-up: zero matmuls keep the activity monitor busy
                # while the first weight/x DMAs land.
                wup = pa.tile([128, 512], bf16, name="wup", tag="wup")
                nc.vector.memset(wup[:], 0.0)
                for _ in range(16):
                    wps = paps.tile([128, 512], f32, name="wps", tag="wps", bufs=2)
                    nc.tensor.matmul(wps[:], wup[:, 0:128], wup[:],
                                     start=True, stop=True)
                # preload the exp table set early (one-time ~2.7us)
                dumm = pa.tile([128, 8], f32, name="dumm", tag="dumm")
                nc.scalar.activation(dumm[:], wup[:, 0:8], AF.Exp)

                xt = [pa.tile([128, T], bf16, name=f"xt{k}", tag=f"xt{k}")
                      for k in range(KC)]
                cos_a = pa.tile([128, T], bf16, name="cos_a", tag="cos_a")
                sin_a = pa.tile([128, T], bf16, name="sin_a", tag="sin_a")
                cos_b = pa.tile([128, T], bf16, name="cos_b", tag="cos_b")
                sin_b = pa.tile([128, T], bf16, name="sin_b", tag="sin_b")

                def qk_phase(wdram, dstA, dstBr, first=False):
                    wsb = []
                    for k in range(KC):
                        wt = pa.tile([128, EP], bf16, name=f"w{k}", tag=f"W{k}")
                        # b-cols first: the B block is projected first
                        nc.sync.dma_start(
                            wt[:, 512:EP], wdram[k * 128:(k + 1) * 128, 512:EP])
                        nc.sync.dma_start(
                            wt[:, 0:512], wdram[k * 128:(k + 1) * 128, 0:512])
                        wsb.append(wt)
                        if first:
                            # x chunk right after its weight chunk, split
                            # across three engine DMA queues: they trigger
                            # independent DMA engines, so the 4.6MB x stream
                            # arrives ~2-3x faster than on the sync queue alone
                            for j, eng in enumerate(
                                    (nc.sync, nc.sync, nc.scalar, nc.gpsimd)):
                                eng.dma_start(
                                    xt[k][:, j * 512:(j + 1) * 512],
                                    xT[k * 128:(k + 1) * 128, j * 512:(j + 1) * 512])

                    def proj_block(cols, dst, sprinkle=False):
                        # dst (bf16 SBUF) <- (wsb[:, cols]).T @ xt
                        # k-outer: one stationary load feeds all 4 t-chains
                        pss = [paps.tile([128, 512], f32, name=f"pps{tg}",
                                         tag=f"projps{tg}", bufs=1)
                               for tg in range(4)]
                        for k in range(KC):
                            for tg in range(4):
                                nc.tensor.matmul(
                                    pss[tg][:], wsb[k][:, cols],
                                    xt[k][:, tg * 512:(tg + 1) * 512],
                                    start=(k == 0), stop=(k == KC - 1))
                            if sprinkle and k % 2 == 0:
                                # dep-free dummy MM: during the DMA-gated ramp
                                # these fill PE idle windows so the HAM clock
                                # gate never re-throttles to K=4/8
                                wps = paps.tile([128, 512], f32, name="wps",
                                                tag="wps", bufs=2)
                                nc.tensor.matmul(wps[:], wup[:, 0:128], wup[:],
                                                 start=True, stop=True)
                        for tg in range(4):
                            nc.scalar.copy(dst[:, tg * 512:(tg + 1) * 512],
                                           pss[tg][:])

                    rawB = pa.tile([128, T], bf16, name="rawB", tag="rawB")
                    proj_block(slice(512, EP), rawB, sprinkle=first)
                    if first:
                        # trig loads delayed behind the B-block evac on the
                        # scalar FIFO so they don't steal HBM bandwidth from
                        # the critical first w/x loads
                        nc.scalar.dma_start(cos_a[:], cosTa[:])
                        nc.scalar.dma_start(sin_a[:], sinTa[:])
                        nc.scalar.dma_start(cos_b[:], cosTb[:])
                        nc.scalar.dma_start(sin_b[:], sinTb[:])
                    qSB = pa.tile([128, T], bf16, name="qSB", tag="qSB")
                    nc.gpsimd.memset(qSB[:], 0.0)

                    for h in range(HL):
                        rawA = pa.tile([128, T], bf16, name="rawA",
                                       tag="rawA", bufs=3)
                        proj_block(slice(h * 128, (h + 1) * 128), rawA,
                                   sprinkle=(first and h < 2))
                        # partition-shifted replica qS[e] = raw[partner(e)]
                        qS = pa.tile([128, T], bf16, name="qS", tag="qS", bufs=2)
                        nc.gpsimd.dma_start(qS[0:56, :], rawA[72:128, :])
                        nc.gpsimd.dma_start(qS[56:72, :], rawB[32 * h:32 * h + 16, :])
                        nc.gpsimd.dma_start(qS[72:128, :], rawA[0:56, :])
                        nc.gpsimd.dma_start(qSB[32 * h:32 * h + 16, :], rawA[56:72, :])
                        m1 = pa.tile([128, T], bf16, name="m1", tag="m1", bufs=2)
                        m2 = pa.tile([128, T], bf16, name="m2", tag="m2", bufs=2)
                        nc.vector.tensor_mul(m1[:], qS[:], sin_a[:])
                        nc.vector.tensor_mul(m2[:], rawA[:], cos_a[:])
                        nc.vector.tensor_add(dstA[h][:], m1[:], m2[:])

                    # b-block rope + 4x row-group replication
                    mB1 = pa.tile([128, T], bf16, name="mB1", tag="m1", bufs=2)
                    mB2 = pa.tile([128, T], bf16, name="mB2", tag="m2", bufs=2)
                    qTB = pa.tile([128, T], bf16, name="qTB", tag="qTB")
                    nc.vector.tensor_mul(mB1[:], qSB[:], sin_b[:])
                    nc.vector.tensor_mul(mB2[:], rawB[:], cos_b[:])
                    nc.vector.tensor_add(qTB[:], mB1[:], mB2[:])
                    for h in range(HL):
                        for j in range(4):
                            nc.gpsimd.dma_start(
                                dstBr[h][32 * j:32 * j + 32, :],
                                qTB[32 * h:32 * h + 32, :])

                # q first, then k, then v: each phase's b-block rope +
                # replication tail is hidden under the next phase's matmuls,
                # so the first attention group starts with all deps ready.
                qk_phase(wqT, qTa, qTBr, first=True)
                qk_phase(wkT, kTa, kTBr)

                # ---- v projection (natural [t, e] layout) ----
                wv_sb = []
                for k in range(KC):
                    wt = pa.tile([128, DV], bf16, name=f"wv{k}", tag=f"W{k}")
                    nc.sync.dma_start(wt[:], wvT[k * 128:(k + 1) * 128, :])
                    wv_sb.append(wt)
                for n in range(NT):
                    pg = 2 * (n % 2)   # alternate tag pairs = double buffering
                    ps0 = paps.tile([128, 288], f32, name="ps0",
                                    tag=f"projps{pg}", bufs=1)
                    ps1 = paps.tile([128, 288], f32, name="ps1",
                                    tag=f"projps{pg + 1}", bufs=1)
                    for k in range(KC):
                        lhs = xt[k][:, n * 128:(n + 1) * 128]
                        nc.tensor.matmul(ps0[:], lhs, wv_sb[k][:, 0:288],
                                         start=(k == 0), stop=(k == KC - 1))
                        nc.tensor.matmul(ps1[:], lhs, wv_sb[k][:, 288:DV],
                                         start=(k == 0), stop=(k == KC - 1))
                    v3 = vt[n].rearrange("p (h e) -> p h e", h=HL)
                    nc.scalar.copy(v3[:, 0:2, 0:HD],
                                   ps0.rearrange("p (h e) -> p h e", h=2))
                    nc.scalar.copy(v3[:, 2:4, 0:HD],
                                   ps1.rearrange("p (h e) -> p h e", h=2))
                    nc.vector.memset(v3[:, :, HD:HD + 1], 1.0)

            # ------------- Phase B: attention ------------------------------
            with tc.tile_pool(name="pb", bufs=1) as pb:
                ot = [pb.tile([128, DV], bf16, name=f"o{t}", tag=f"o{t}")
                      for t in range(NT)]
                with tc.tile_pool(name="pbps", bufs=1, space="PSUM") as pbps:
                    for qb in range(4):
                        for h in range(HL):
                            o_ps3 = pbps.tile([128, 3 * (HD + 1)], f32,
                                              name="o_ps3", tag="o3", bufs=2)
                            o_ps1 = pbps.tile([128, HD + 1], f32,
                                              name="o_ps1", tag="o1", bufs=2)
                            o_ps = [
                                o_ps3[:, 0:HD + 1],
                                o_ps3[:, HD + 1:2 * (HD + 1)],
                                o_ps3[:, 2 * (HD + 1):3 * (HD + 1)],
                                o_ps1[:],
                            ]

                            # key-tile groups of 2: one [128,1024] score-PSUM
                            # tile (2 banks) per group -> one Exp covers 1024
                            # elems/partition; with o3/o1 double-buffered the
                            # PSUM budget is 4+2+2 = 8 banks exactly
                            GRPS = [(2 * i, 2) for i in range(8)]

                            def s_exp(g):
                                kt0, gn = GRPS[g]
                                sps = pbps.tile([128, 1024], f32, name="sps",
                                                tag="sc", bufs=2)
                                # K=32 b-block tails first (start=True clears
                                # the bank), so the K=128 mains run
                                # back-to-back with stop=True
                                for j in range(gn):
                                    kt = kt0 + j
                                    rg = kt % 4
                                    nc.tensor.matmul(
                                        sps[:, j * 512:(j + 1) * 512],
                                        kTBr[h][32 * rg:32 * rg + 32,
                                                kt * 128:(kt + 1) * 128],
                                        qTBr[h][32 * rg:32 * rg + 32,
                                                qb * 512:(qb + 1) * 512],
                                        start=True, stop=False,
                                        tile_position=(32 * rg, 0))
                                for j in range(gn):
                                    kt = kt0 + j
                                    nc.tensor.matmul(
                                        sps[:, j * 512:(j + 1) * 512],
                                        kTa[h][:, kt * 128:(kt + 1) * 128],
                                        qTa[h][:, qb * 512:(qb + 1) * 512],
                                        start=False, stop=True)
                                E = pb.tile([128, 1024], bf16, name="E",
                                            tag="E", bufs=4)
                                nc.scalar.activation(E[:, 0:gn * 512],
                                                     sps[:, 0:gn * 512],
                                                     AF.Exp, scale=SCALE)
                                return E

                            def pv(g, E):
                                kt0, gn = GRPS[g]
                                for j in range(gn):
                                    kt = kt0 + j
                                    for qt in range(4):
                                        if qt < 3:
                                            st = kt == 0 and qt == 0
                                            sp = kt == NT - 1 and qt == 2
                                        else:
                                            st = kt == 0
                                            sp = kt == NT - 1
                                        nc.tensor.matmul(
                                            o_ps[qt][:],
                                            E[:, j * 512 + qt * 128:
                                              j * 512 + (qt + 1) * 128],
                                            vt[kt][:, (HD + 1) * h:
                                                   (HD + 1) * (h + 1)],
                                            start=st, stop=sp)

                            ngrp = len(GRPS)
                            Ep = s_exp(0)
                            for g in range(ngrp):
                                En = s_exp(g + 1) if g + 1 < ngrp else None
                                pv(g, Ep)
                                Ep = En
                            for qt in range(4):
                                t = qb * 4 + qt
                                r = pb.tile([128, 1], f32, name="r", tag="r",
                                            bufs=4)
                                nc.vector.reciprocal(r[:], o_ps[qt][:, HD:HD + 1])
                                nc.vector.tensor_scalar_mul(
                                    ot[t][:, HD * h:HD * (h + 1)],
                                    o_ps[qt][:, 0:HD], r[:])

                # ------------- Phase C: o^T + final projection -------------
                oTa = [pb.tile([128, T], bf16, name=f"oTa{j}", tag=f"oTa{j}")
                       for j in range(4)]
                oTb = pb.tile([64, T], bf16, name="oTb", tag="oTb")
                wo_sb = []
                for k in range(5):
                    rows = 128 if k < 4 else 64
                    wot = pb.tile([128, D], bf16, name=f"wo{k}", tag=f"wo{k}")
                    nc.sync.dma_start(wot[0:rows, :], woT[k * 128:k * 128 + rows, :])
                    wo_sb.append(wot)
                with tc.tile_pool(name="pcps", bufs=1, space="PSUM") as pcps:

                    def o_transp(t):
                        for j in range(4):
                            tp = pcps.tile([128, 128], bf16, name="tpo",
                                           tag="otp", bufs=4)
                            nc.tensor.transpose(
                                tp[:], ot[t][:, 128 * j:128 * (j + 1)],
                                ident_bf[:])
                            nc.scalar.copy(oTa[j][:, t * 128:(t + 1) * 128], tp[:])
                        tpb = pcps.tile([64, 128], bf16, name="tpb",
                                        tag="otp", bufs=4)
                        nc.tensor.transpose(tpb[:], ot[t][:, 512:DV], ident_bf[:])
                        nc.scalar.copy(oTb[:, t * 128:(t + 1) * 128], tpb[:])

                    def final(t):
                        for j3 in range(3):
                            fps = pcps.tile([128, 384], f32, name="fps",
                                            tag="f", bufs=4)
                            for k in range(5):
                                lhs = (oTa[k][:, t * 128:(t + 1) * 128]
                                       if k < 4
                                       else oTb[:, t * 128:(t + 1) * 128])
                                nc.tensor.matmul(
                                    fps[:], lhs,
                                    wo_sb[k][0:(128 if k < 4 else 64),
                                             384 * j3:384 * (j3 + 1)],
                                    start=(k == 0), stop=(k == 4))
                            fout = pb.tile([128, 384], f32, name="fout",
                                           tag="fout", bufs=6)
                            nc.vector.tensor_copy(fout[:], fps[:])
                            nc.sync.dma_start(
                                out[t * 128:(t + 1) * 128,
                                    384 * j3:384 * (j3 + 1)], fout[:])

                    o_transp(0)
                    for t in range(NT):
                        if t + 1 < NT:
                            o_transp(t + 1)
                        final(t)

    nc.compile()
    return nc


def get_nc(debug=False):
    key = bool(debug)
    if key not in _NC_CACHE:
        _NC_CACHE[key] = _build(debug)
    return _NC_CACHE[key]


def make_in_maps(x, cos, sin, Wq, Wk, Wv, Wo):
    import ml_dtypes

    x = np.asarray(x, np.float32)
    cos = np.asarray(cos, np.float32)
    sin = np.asarray(sin, np.float32)
    Wq, Wk, Wv, Wo = (np.asarray(w, np.float32) for w in (Wq, Wk, Wv, Wo))

    # transposed trig tables with the rotate-half signs folded in:
    # out[e] = raw[e]*cos[e] + sgn(e)*raw[partner(e)]*sin[e]
    cosT = np.ascontiguousarray(cos.T)   # [144, T]
    sinT = np.ascontiguousarray(sin.T)
    sgn = np.ones((128, 1), np.float32)
    sgn[:72] = -1.0
    cosTa = cosT[0:128]
    sinTa = sinT[0:128] * sgn
    cosTb = np.zeros((128, T), np.float32)
    sinTb = np.zeros((128, T), np.float32)
    for hh in range(HL):
        cosTb[32 * hh:32 * hh + 16] = cosT[128:144]
        sinTb[32 * hh:32 * hh + 16] = sinT[128:144]
    bf = ml_dtypes.bfloat16

    in_maps = []
    for c in range(NCORES):
        b, hg = divmod(c, 2)
        heads = [HL * hg + i for i in range(HL)]

        def qk_w(W):
            Wsel = np.zeros((EP, D), np.float32)
            for i, g in enumerate(heads):
                Wsel[128 * i:128 * i + 128] = W[144 * g:144 * g + 128]
                Wsel[512 + 32 * i:512 + 32 * i + 16] = W[144 * g + 128:144 * g + 144]
            return np.ascontiguousarray(Wsel.T)

        wv_sel = np.concatenate([Wv[144 * g:144 * g + 144] for g in heads], 0)
        wo_sel = np.concatenate([Wo[:, 144 * g:144 * g + 144] for g in heads], 1)
        in_maps.append(
            {
                "xT": np.ascontiguousarray(x[b].T).astype(bf),
                "wqT": qk_w(Wq).astype(bf),
                "wkT": qk_w(Wk).astype(bf),
                "wvT": np.ascontiguousarray(wv_sel.T).astype(bf),
                "woT": np.ascontiguousarray(wo_sel.T).astype(bf),
                "cosTa": cosTa.astype(bf),
                "sinTa": sinTa.astype(bf),
                "cosTb": cosTb.astype(bf),
                "sinTb": sinTb.astype(bf),
                "identB": np.eye(128, dtype=bf),
            }
        )
    return in_maps


def kernel(x, cos, sin, Wq, Wk, Wv, Wo, _trace=False, _trace_kwargs=None):
    from concourse.bass_utils import run_bass_kernel_spmd

    nc = get_nc()
    in_maps = make_in_maps(x, cos, sin, Wq, Wk, Wv, Wo)
    res = run_bass_kernel_spmd(
        nc,
        in_maps,
        list(range(NCORES)),
        trace=_trace,
        **(_trace_kwargs or {}),
    )
    parts = [res.results[c]["out"] for c in range(NCORES)]
    outb = np.stack([parts[2 * b] + parts[2 * b + 1] for b in range(B)])
    if _trace:
        kernel.last_results = res
    return outb.astype(np.float32)


# revision 26
# speedup vs baseline: 1.1806x; 1.0177x over previous
"""Trainium2 Bass kernel for a fused multi-head attention block.

Reference computation (B=4, T=2048, D=1152, H=8, HD=144, full rotary):
    q,k,v = x@Wq.T, x@Wk.T, x@Wv.T   (per head)
    q,k   = rope(q, k, cos, sin)
    o     = softmax(q k^T / sqrt(HD)) v
    out   = o @ Wo.T
Sharding (8 cores): core c = (batch b = c//2, head-group hg = c%2).
Each core computes 4 heads of one batch and a partial output
out_part = o_local @ Wo[:, hg_cols].T ; host sums the two partials per batch.

v2 design (vs. the transpose-heavy v1):
  * q/k are projected DIRECTLY transposed: qT = Wsel^T-chunks (stationary)
    x xT (moving), so the scores layout [dim, T] needs no PE transposes.
  * rope runs in the transposed layout: the rotate-half partner lives at a
    partition offset, so a DMA SBUF->SBUF copy builds a partition-shifted
    replica qS with qS[e] = q[partner(e)]; then
    q_rot = q * cosT + qS * sinT_signed  (3 bf16 tensor_tensor ops, the
    rotate-half signs are folded into sinT_signed on the host).
  * per-head packing (same as v1): heads' dims 0..127 in four [128,T] tiles,
    dims 128..143 in a shared b-block tile at rows 32h..32h+16 (+16 zero pad),
    replicated to all four 32-row groups so the K=32 score-tail matmuls can
    run concurrently via tile_position.
  * scores S^T accumulate in a [128,1024] PSUM tile (two banks, two
    key-tiles per group) so each Exp activation covers 1024 elems/partition —
    halving ScalarE instruction overhead vs. [128,512] activations.
  * softmax denominator via ones-column appended to v (o_ps[:,144]).
  * phase C (oT transpose + final projection) runs fully in bf16 and all
    PSUM->SBUF evacuations go to Vector/Scalar (never the same engine that
    gates the PE), keeping the PE stream dense so HAM stays at K=8/8.
  * 16 zero matmuls at t=0 prime the PE activity monitor while the first
    DMAs land; a dummy Exp preloads the ACT table set.
"""

import numpy as np

B, T, D, H = 4, 2048, 1152, 8
HL = 4              # heads per core
HD = 144            # head dim
EP = 640            # packed q/k width: 4*128 + 128 (4x(16+16pad))
DV = HL * HD        # 576, v/o width
VW = HL * (HD + 1)  # 580, v + ones col
NT = T // 128       # 16 t-tiles
KC = D // 128       # 9 contraction chunks
SCALE = float(HD) ** -0.5
NCORES = 8

_NC_CACHE = {}


def _build(debug=False):
    import concourse.bacc as bacc
    import concourse.mybir as mybir
    from concourse.tile import TileContext

    dt = mybir.dt
    f32, bf16 = dt.float32, dt.bfloat16
    AF = mybir.ActivationFunctionType

    nc = bacc.Bacc(
        "TRN2",
        target_bir_lowering=False,
        debug=debug,
        enable_asserts=False,
        num_devices=NCORES,
    )

    xT = nc.declare_dram_parameter("xT", [D, T], bf16, isOutput=False)
    wqT = nc.declare_dram_parameter("wqT", [D, EP], bf16, isOutput=False)
    wkT = nc.declare_dram_parameter("wkT", [D, EP], bf16, isOutput=False)
    wvT = nc.declare_dram_parameter("wvT", [D, DV], bf16, isOutput=False)
    woT = nc.declare_dram_parameter("woT", [DV, D], bf16, isOutput=False)
    cosTa = nc.declare_dram_parameter("cosTa", [128, T], bf16, isOutput=False)
    sinTa = nc.declare_dram_parameter("sinTa", [128, T], bf16, isOutput=False)
    cosTb = nc.declare_dram_parameter("cosTb", [128, T], bf16, isOutput=False)
    sinTb = nc.declare_dram_parameter("sinTb", [128, T], bf16, isOutput=False)
    identB = nc.declare_dram_parameter("identB", [128, 128], bf16, isOutput=False)
    out = nc.declare_dram_parameter("out", [T, D], f32, isOutput=True)

    with TileContext(nc) as tc:
        with tc.tile_pool(name="persist", bufs=1) as P0:
            ident_bf = P0.tile([128, 128], bf16, name="ident_bf", tag="ident_bf")
            nc.sync.dma_start(ident_bf[:], identB[:])

            qTa = [P0.tile([128, T], bf16, name=f"qTa{h}", tag=f"qTa{h}")
                   for h in range(HL)]
            kTa = [P0.tile([128, T], bf16, name=f"kTa{h}", tag=f"kTa{h}")
                   for h in range(HL)]
            qTBr = [P0.tile([128, T], bf16, name=f"qTBr{h}", tag=f"qTBr{h}")
                    for h in range(HL)]
            kTBr = [P0.tile([128, T], bf16, name=f"kTBr{h}", tag=f"kTBr{h}")
                    for h in range(HL)]
            vt = [P0.tile([128, VW], bf16, name=f"v{t}", tag=f"v{t}")
                  for t in range(NT)]

            # ------------- Phase A: transposed projections + rope ----------
            with (
                tc.tile_pool(name="pa", bufs=1) as pa,
                tc.tile_pool(name="paps", bufs=1, space="PSUM") as paps,
            ):
                # PE warm-up: zero matmuls keep the activity monitor busy
                # while the first weight/x DMAs land.
                wup = pa.tile([128, 512], bf16, name="wup", tag="wup")
                nc.vector.memset(wup[:], 0.0)
                for _ in range(16):
                    wps = paps.tile([128, 512], f32, name="wps", tag="wps", bufs=2)
                    nc.tensor.matmul(wps[:], wup[:, 0:128], wup[:],
                                     start=True, stop=True)
                # preload the exp table set early (one-time ~2.7us)
                dumm = pa.tile([128, 8], f32, name="dumm", tag="dumm")
                nc.scalar.activation(dumm[:], wup[:, 0:8], AF.Exp)

                xt = [pa.tile([128, T], bf16, name=f"xt{k}", tag=f"xt{k}")
                      for k in range(KC)]
                cos_a = pa.tile([128, T], bf16, name="cos_a", tag="cos_a")
                sin_a = pa.tile([128, T], bf16, name="sin_a", tag="sin_a")
                cos_b = pa.tile([128, T], bf16, name="cos_b", tag="cos_b")
                sin_b = pa.tile([128, T], bf16, name="sin_b", tag="sin_b")

                def qk_phase(wdram, dstA, dstBr, first=False):
                    wsb = []
                    for k in range(KC):
                        wt = pa.tile([128, EP], bf16, name=f"w{k}", tag=f"W{k}")
                        # b-cols first: the B block is projected first
                        nc.sync.dma_start(
                            wt[:, 512:EP], wdram[k * 128:(k + 1) * 128, 512:EP])
                        if not first:
                            nc.sync.dma_start(
                                wt[:, 0:512], wdram[k * 128:(k + 1) * 128, 0:512])
                        wsb.append(wt)
                        if first:
                            # x chunk right after its weight chunk, balanced
                            # across three engine DMA queues (~2.3MB each):
                            # they trigger independent DMA engines, so the
                            # 6MB w+x input stream arrives ~2-3x faster than
                            # on the sync queue alone
                            for j, eng in enumerate(
                                    (nc.sync, nc.scalar, nc.gpsimd, nc.gpsimd)):
                                eng.dma_start(
                                    xt[k][:, j * 512:(j + 1) * 512],
                                    xT[k * 128:(k + 1) * 128, j * 512:(j + 1) * 512])
                    if first:
                        # w main-cols after the tg1 sweep on the scalar queue;
                        # needed only when the h-blocks start (~16us in)
                        for k in range(KC):
                            nc.scalar.dma_start(
                                wsb[k][:, 0:512],
                                wdram[k * 128:(k + 1) * 128, 0:512])

                    def proj_block(cols, dst, sprinkle=False):
                        # dst (bf16 SBUF) <- (wsb[:, cols]).T @ xt
                        # k-outer: one stationary load feeds all 4 t-chains
                        pss = [paps.tile([128, 512], f32, name=f"pps{tg}",
                                         tag=f"projps{tg}", bufs=1)
                               for tg in range(4)]
                        for k in range(KC):
                            for tg in range(4):
                                nc.tensor.matmul(
                                    pss[tg][:], wsb[k][:, cols],
                                    xt[k][:, tg * 512:(tg + 1) * 512],
                                    start=(k == 0), stop=(k == KC - 1))
                            if sprinkle:
                                # dep-free dummy MM: during the DMA-gated ramp
                                # these fill PE idle windows so the HAM clock
                                # gate never re-throttles to K=4/8
                                wps = paps.tile([128, 512], f32, name="wps",
                                                tag="wps", bufs=2)
                                nc.tensor.matmul(wps[:], wup[:, 0:128], wup[:],
                                                 start=True, stop=True)
                        for tg in range(4):
                            nc.scalar.copy(dst[:, tg * 512:(tg + 1) * 512],
                                           pss[tg][:])

                    rawB = pa.tile([128, T], bf16, name="rawB", tag="rawB")
                    proj_block(slice(512, EP), rawB, sprinkle=first)
                    if first:
                        # trig loads delayed behind the B-block evac on the
                        # scalar FIFO so they don't steal HBM bandwidth from
                        # the critical first w/x loads
                        nc.scalar.dma_start(cos_a[:], cosTa[:])
                        nc.scalar.dma_start(sin_a[:], sinTa[:])
                        nc.scalar.dma_start(cos_b[:], cosTb[:])
                        nc.scalar.dma_start(sin_b[:], sinTb[:])
                    qSB = pa.tile([128, T], bf16, name="qSB", tag="qSB")
                    nc.gpsimd.memset(qSB[:], 0.0)

                    for h in range(HL):
                        rawA = pa.tile([128, T], bf16, name="rawA",
                                       tag="rawA", bufs=3)
                        proj_block(slice(h * 128, (h + 1) * 128), rawA,
                                   sprinkle=(first and h < 3))
                        # partition-shifted replica qS[e] = raw[partner(e)]
                        qS = pa.tile([128, T], bf16, name="qS", tag="qS", bufs=2)
                        nc.gpsimd.dma_start(qS[0:56, :], rawA[72:128, :])
                        nc.gpsimd.dma_start(qS[56:72, :], rawB[32 * h:32 * h + 16, :])
                        nc.gpsimd.dma_start(qS[72:128, :], rawA[0:56, :])
                        nc.gpsimd.dma_start(qSB[32 * h:32 * h + 16, :], rawA[56:72, :])
                        m1 = pa.tile([128, T], bf16, name="m1", tag="m1", bufs=2)
                        m2 = pa.tile([128, T], bf16, name="m2", tag="m2", bufs=2)
                        nc.vector.tensor_mul(m1[:], qS[:], sin_a[:])
                        nc.vector.tensor_mul(m2[:], rawA[:], cos_a[:])
                        nc.vector.tensor_add(dstA[h][:], m1[:], m2[:])

                    # b-block rope + 4x row-group replication
                    mB1 = pa.tile([128, T], bf16, name="mB1", tag="m1", bufs=2)
                    mB2 = pa.tile([128, T], bf16, name="mB2", tag="m2", bufs=2)
                    qTB = pa.tile([128, T], bf16, name="qTB", tag="qTB")
                    nc.vector.tensor_mul(mB1[:], qSB[:], sin_b[:])
                    nc.vector.tensor_mul(mB2[:], rawB[:], cos_b[:])
                    nc.vector.tensor_add(qTB[:], mB1[:], mB2[:])
                    for h in range(HL):
                        for j in range(4):
                            nc.gpsimd.dma_start(
                                dstBr[h][32 * j:32 * j + 32, :],
                                qTB[32 * h:32 * h + 32, :])

                # q first, then k, then v: each phase's b-block rope +
                # replication tail is hidden under the next phase's matmuls,
                # so the first attention group starts with all deps ready.
                qk_phase(wqT, qTa, qTBr, first=True)
                qk_phase(wkT, kTa, kTBr)

                # ---- v projection (natural [t, e] layout) ----
                wv_sb = []
                for k in range(KC):
                    wt = pa.tile([128, DV], bf16, name=f"wv{k}", tag=f"W{k}")
                    nc.sync.dma_start(wt[:], wvT[k * 128:(k + 1) * 128, :])
                    wv_sb.append(wt)
                for n in range(NT):
                    pg = 2 * (n % 2)   # alternate tag pairs = double buffering
                    ps0 = paps.tile([128, 288], f32, name="ps0",
                                    tag=f"projps{pg}", bufs=1)
                    ps1 = paps.tile([128, 288], f32, name="ps1",
                                    tag=f"projps{pg + 1}", bufs=1)
                    for k in range(KC):
                        lhs = xt[k][:, n * 128:(n + 1) * 128]
                        nc.tensor.matmul(ps0[:], lhs, wv_sb[k][:, 0:288],
                                         start=(k == 0), stop=(k == KC - 1))
                        nc.tensor.matmul(ps1[:], lhs, wv_sb[k][:, 288:DV],
                                         start=(k == 0), stop=(k == KC - 1))
                    v3 = vt[n].rearrange("p (h e) -> p h e", h=HL)
                    nc.scalar.copy(v3[:, 0:2, 0:HD],
                                   ps0.rearrange("p (h e) -> p h e", h=2))
                    nc.scalar.copy(v3[:, 2:4, 0:HD],
                                   ps1.rearrange("p (h e) -> p h e", h=2))
                    nc.vector.memset(v3[:, :, HD:HD + 1], 1.0)

            # ------------- Phase B: attention ------------------------------
            with tc.tile_pool(name="pb", bufs=1) as pb:
                ot = [pb.tile([128, DV], bf16, name=f"o{t}", tag=f"o{t}")
                      for t in range(NT)]
                with tc.tile_pool(name="pbps", bufs=1, space="PSUM") as pbps:
                    for qb in range(4):
                        for h in range(HL):
                            o_ps3 = pbps.tile([128, 3 * (HD + 1)], f32,
                                              name="o_ps3", tag="o3", bufs=2)
                            o_ps1 = pbps.tile([128, HD + 1], f32,
                                              name="o_ps1", tag="o1", bufs=2)
                            o_ps = [
                                o_ps3[:, 0:HD + 1],
                                o_ps3[:, HD + 1:2 * (HD + 1)],
                                o_ps3[:, 2 * (HD + 1):3 * (HD + 1)],
                                o_ps1[:],
                            ]

                            # key-tile groups of 2: one [128,1024] score-PSUM
                            # tile (2 banks) per group -> one Exp covers 1024
                            # elems/partition; with o3/o1 double-buffered the
                            # PSUM budget is 4+2+2 = 8 banks exactly
                            GRPS = [(2 * i, 2) for i in range(8)]

                            def s_exp(g):
                                kt0, gn = GRPS[g]
                                sps = pbps.tile([128, 1024], f32, name="sps",
                                                tag="sc", bufs=2)
                                # K=32 b-block tails first (start=True clears
                                # the bank), so the K=128 mains run
                                # back-to-back with stop=True
                                for j in range(gn):
                                    kt = kt0 + j
                                    rg = kt % 4
                                    nc.tensor.matmul(
                                        sps[:, j * 512:(j + 1) * 512],
                                        kTBr[h][32 * rg:32 * rg + 32,
                                                kt * 128:(kt + 1) * 128],
                                        qTBr[h][32 * rg:32 * rg + 32,
                                                qb * 512:(qb + 1) * 512],
                                        start=True, stop=False,
                                        tile_position=(32 * rg, 0))
                                for j in range(gn):
                                    kt = kt0 + j
                                    nc.tensor.matmul(
                                        sps[:, j * 512:(j + 1) * 512],
                                        kTa[h][:, kt * 128:(kt + 1) * 128],
                                        qTa[h][:, qb * 512:(qb + 1) * 512],
                                        start=False, stop=True)
                                E = pb.tile([128, 1024], bf16, name="E",
                                            tag="E", bufs=4)
                                nc.scalar.activation(E[:, 0:gn * 512],
                                                     sps[:, 0:gn * 512],
                                                     AF.Exp, scale=SCALE)
                                return E

                            def pv(g, E):
                                kt0, gn = GRPS[g]
                                for j in range(gn):
                                    kt = kt0 + j
                                    for qt in range(4):
                                        if qt < 3:
                                            st = kt == 0 and qt == 0
                                            sp = kt == NT - 1 and qt == 2
                                        else:
                                            st = kt == 0
                                            sp = kt == NT - 1
                                        nc.tensor.matmul(
                                            o_ps[qt][:],
                                            E[:, j * 512 + qt * 128:
                                              j * 512 + (qt + 1) * 128],
                                            vt[kt][:, (HD + 1) * h:
                                                   (HD + 1) * (h + 1)],
                                            start=st, stop=sp)

                            ngrp = len(GRPS)
                            Ep = s_exp(0)
                            for g in range(ngrp):
                                En = s_exp(g + 1) if g + 1 < ngrp else None
                                pv(g, Ep)
                                Ep = En
                            for qt in range(4):
                                t = qb * 4 + qt
                                r = pb.tile([128, 1], f32, name="r", tag="r",
                                            bufs=4)
                                nc.vector.reciprocal(r[:], o_ps[qt][:, HD:HD + 1])
                                nc.vector.tensor_scalar_mul(
                                    ot[t][:, HD * h:HD * (h + 1)],
                                    o_ps[qt][:, 0:HD], r[:])

                # ------------- Phase C: o^T + final projection -------------
                oTa = [pb.tile([128, T], bf16, name=f"oTa{j}", tag=f"oTa{j}")
                       for j in range(4)]
                oTb = pb.tile([64, T], bf16, name="oTb", tag="oTb")
                wo_sb = []
                for k in range(5):
                    rows = 128 if k < 4 else 64
                    wot = pb.tile([128, D], bf16, name=f"wo{k}", tag=f"wo{k}")
                    nc.sync.dma_start(wot[0:rows, :], woT[k * 128:k * 128 + rows, :])
                    wo_sb.append(wot)
                with tc.tile_pool(name="pcps", bufs=1, space="PSUM") as pcps:

                    def o_transp(t):
                        for j in range(4):
                            tp = pcps.tile([128, 128], bf16, name="tpo",
                                           tag="otp", bufs=4)
                            nc.tensor.transpose(
                                tp[:], ot[t][:, 128 * j:128 * (j + 1)],
                                ident_bf[:])
                            nc.scalar.copy(oTa[j][:, t * 128:(t + 1) * 128], tp[:])
                        tpb = pcps.tile([64, 128], bf16, name="tpb",
                                        tag="otp", bufs=4)
                        nc.tensor.transpose(tpb[:], ot[t][:, 512:DV], ident_bf[:])
                        nc.scalar.copy(oTb[:, t * 128:(t + 1) * 128], tpb[:])

                    def final(t):
                        for j3 in range(3):
                            fps = pcps.tile([128, 384], f32, name="fps",
                                            tag="f", bufs=4)
                            for k in range(5):
                                lhs = (oTa[k][:, t * 128:(t + 1) * 128]
                                       if k < 4
                                       else oTb[:, t * 128:(t + 1) * 128])
                                nc.tensor.matmul(
                                    fps[:], lhs,
                                    wo_sb[k][0:(128 if k < 4 else 64),
                                             384 * j3:384 * (j3 + 1)],
                                    start=(k == 0), stop=(k == 4))
                            fout = pb.tile([128, 384], f32, name="fout",
                                           tag="fout", bufs=6)
                            nc.vector.tensor_copy(fout[:], fps[:])
                            nc.sync.dma_start(
                                out[t * 128:(t + 1) * 128,
                                    384 * j3:384 * (j3 + 1)], fout[:])

                    o_transp(0)
                    for t in range(NT):
                        if t + 1 < NT:
                            o_transp(t + 1)
                        final(t)

    nc.compile()
    return nc


def get_nc(debug=False):
    key = bool(debug)
    if key not in _NC_CACHE:
        _NC_CACHE[key] = _build(debug)
    return _NC_CACHE[key]


def make_in_maps(x, cos, sin, Wq, Wk, Wv, Wo):
    import ml_dtypes

    x = np.asarray(x, np.float32)
    cos = np.asarray(cos, np.float32)
    sin = np.asarray(sin, np.float32)
    Wq, Wk, Wv, Wo = (np.asarray(w, np.float32) for w in (Wq, Wk, Wv, Wo))

    # transposed trig tables with the rotate-half signs folded in:
    # out[e] = raw[e]*cos[e] + sgn(e)*raw[partner(e)]*sin[e]
    cosT = np.ascontiguousarray(cos.T)   # [144, T]
    sinT = np.ascontiguousarray(sin.T)
    sgn = np.ones((128, 1), np.float32)
    sgn[:72] = -1.0
    cosTa = cosT[0:128]
    sinTa = sinT[0:128] * sgn
    cosTb = np.zeros((128, T), np.float32)
    sinTb = np.zeros((128, T), np.float32)
    for hh in range(HL):
        cosTb[32 * hh:32 * hh + 16] = cosT[128:144]
        sinTb[32 * hh:32 * hh + 16] = sinT[128:144]
    bf = ml_dtypes.bfloat16

    in_maps = []
    for c in range(NCORES):
        b, hg = divmod(c, 2)
        heads = [HL * hg + i for i in range(HL)]

        def qk_w(W):
            Wsel = np.zeros((EP, D), np.float32)
            for i, g in enumerate(heads):
                Wsel[128 * i:128 * i + 128] = W[144 * g:144 * g + 128]
                Wsel[512 + 32 * i:512 + 32 * i + 16] = W[144 * g + 128:144 * g + 144]
            return np.ascontiguousarray(Wsel.T)

        wv_sel = np.concatenate([Wv[144 * g:144 * g + 144] for g in heads], 0)
        wo_sel = np.concatenate([Wo[:, 144 * g:144 * g + 144] for g in heads], 1)
        in_maps.append(
            {
                "xT": np.ascontiguousarray(x[b].T).astype(bf),
                "wqT": qk_w(Wq).astype(bf),
                "wkT": qk_w(Wk).astype(bf),
                "wvT": np.ascontiguousarray(wv_sel.T).astype(bf),
                "woT": np.ascontiguousarray(wo_sel.T).astype(bf),
                "cosTa": cosTa.astype(bf),
                "sinTa": sinTa.astype(bf),
                "cosTb": cosTb.astype(bf),
                "sinTb": sinTb.astype(bf),
                "identB": np.eye(128, dtype=bf),
            }
        )
    return in_maps


def kernel(x, cos, sin, Wq, Wk, Wv, Wo, _trace=False, _trace_kwargs=None):
    from concourse.bass_utils import run_bass_kernel_spmd

    nc = get_nc()
    in_maps = make_in_maps(x, cos, sin, Wq, Wk, Wv, Wo)
    res = run_bass_kernel_spmd(
        nc,
        in_maps,
        list(range(NCORES)),
        trace=_trace,
        **(_trace_kwargs or {}),
    )
    parts = [res.results[c]["out"] for c in range(NCORES)]
    outb = np.stack([parts[2 * b] + parts[2 * b + 1] for b in range(B)])
    if _trace:
        kernel.last_results = res
    return outb.astype(np.float32)
